# revision 7
# baseline (speedup 1.0000x reference)
"""Trainium2 Bass kernel for nn_Classifier_8461085573484 (2-layer GCN classifier).

Math: with x [N,1] and b1=0 (structurally true for this problem), both GCN
layers collapse to scalar per-node quantities:
  deg_d = indeg(d)+1;  dinv = 1/sqrt(deg);  u = x*dinv
  S_d   = sum_{e->d} u[src];   y = dinv^2*(S + u)   (y = layer1-scalar * dinv)
  sv_d  = sum_{e->d} y[src];  sp_d = sum_{e->d} relu(y[src]);  sm = sp - sv
  alpha = dinv*(sp + relu(y));      beta = dinv*(sm + relu(-y))
  out2  = relu(alpha a^T + beta b^T + b2), a = relu(W1)@W2, b = relu(-W1)@W2
  logits = mean(out2) @ Wl + bl -> log_softmax.

Sharding (8 NeuronCores): nodes are sorted by max(indeg, outdeg) and dealt
round-robin in groups of 1792 to (core, q-chunk); node slot (k, lane, q).
This makes each q-chunk degree-homogeneous, so the padded per-node edge
segments can use a per-chunk width = that chunk's max degree (~1.1x total
slots vs uniform-max padding ~1.7x), and gives every core identical DMA/
compute volume.

Layout: per-edge values live in *expanded row layout* grids [P, cols]:
node (lane, q) owns the KD_c-slot column segment at colD[q]; edge j of that
node (rank by dst or src) occupies slot j, pad slots are zero.  Segment sums
are then plain vector tensor_reduce over [P, QC, KD_c] (no one-hot work
blowup); the neighbor "gather" is a device-side broadcast of the node grid
into the src-major expanded layout (GpSimd engine, overlapping the Vector
reduces).  The host only routes / permutes per-edge values between the
src-major and dst-major layouts (no arithmetic) and applies the O(1)
classifier head.
"""
import contextlib
import ctypes
import sys
import types

import numpy as np

from concourse import bacc, bass, mybir
import concourse.tile as tile
from concourse import bass_utils

P = 128
Q = 98
NSH = P * Q            # 12544 nodes per NC shard
NC = 8
NPAD = NSH * NC        # 100352
N = 100000
F32 = mybir.dt.float32
BF16 = mybir.dt.bfloat16
QC = 14                # q-chunk size (Q = 7 chunks of 14)
NCH = Q // QC          # 7 chunks
G = P * QC             # 1792 nodes per (core, chunk)
EXCH_BF16 = True       # exchange per-edge values in bf16 (halves DMA traffic)


def _install_ntff_shim():
    """Provide antenv.axon_hooks so run_bass_kernel_spmd(trace=True) works."""
    if "antenv.axon_hooks" in sys.modules:
        return
    import antenv

    _hook = None
    try:
        lib = ctypes.CDLL("/opt/axon/libaxon_pjrt.so")
        if hasattr(lib, "axon_start_nrt_profile"):
            lib.axon_start_nrt_profile.argtypes = [
                ctypes.POINTER(ctypes.c_int64), ctypes.c_size_t]
            lib.axon_start_nrt_profile.restype = ctypes.c_int64
            lib.axon_stop_nrt_profile.argtypes = [ctypes.c_char_p]
            lib.axon_stop_nrt_profile.restype = ctypes.c_int64

            @contextlib.contextmanager
            def _hook_impl(output_dir, device_ids):
                import jax
                jax.devices()
                if device_ids:
                    ids = (ctypes.c_int64 * len(device_ids))(*device_ids)
                    rc = lib.axon_start_nrt_profile(ids, len(device_ids))
                else:
                    rc = lib.axon_start_nrt_profile(None, 0)
                if rc != 0:
                    raise RuntimeError(f"axon_start_nrt_profile rc={rc}")
                try:
                    yield
                finally:
                    n = lib.axon_stop_nrt_profile(str(output_dir).encode())
                    if n < 0:
                        raise RuntimeError(f"axon_stop_nrt_profile rc={n}")

            _hook = _hook_impl
    except OSError:
        pass

    mod = types.ModuleType("antenv.axon_hooks")
    mod._hook = _hook
    mod.get_axon_ntff_profile_hook = lambda: mod._hook

    def set_axon_ntff_profile_hook(h):
        mod._hook = h

    mod.set_axon_ntff_profile_hook = set_axon_ntff_profile_hook
    sys.modules["antenv.axon_hooks"] = mod
    antenv.axon_hooks = mod


_install_ntff_shim()


# ---------------- host routing (sharding/layout only, no arithmetic) -------

def _ranks(keys):
    """Rank of each edge within its node group."""
    counts = np.bincount(keys, minlength=NPAD).astype(np.int64)
    starts = np.zeros(NPAD, np.int64)
    starts[1:] = np.cumsum(counts)[:-1]
    order = np.argsort(keys, kind="stable")
    rank = np.empty(keys.shape[0], np.int64)
    rank[order] = np.arange(keys.shape[0], dtype=np.int64) - starts[keys[order]]
    return rank


def _node_layout(indeg, outdeg):
    """Degree-sorted node placement + per-chunk segment widths."""
    key = np.maximum(indeg, outdeg)
    order = np.argsort(-key, kind="stable")        # sorted pos -> node id
    i = np.arange(NPAD)
    g = i // G
    j = i - g * G
    core = g % NC
    chunk = g // NC
    lane = j % P
    q = chunk * QC + (j // P)
    core_of = np.empty(NPAD, np.int64)
    lane_of = np.empty(NPAD, np.int64)
    q_of = np.empty(NPAD, np.int64)
    core_of[order] = core
    lane_of[order] = lane
    q_of[order] = q
    sd = indeg[order].reshape(NCH, NC * G)
    so = outdeg[order].reshape(NCH, NC * G)
    KDs = [int(w) + (int(w) & 1) for w in sd.max(axis=1)]
    KSs = [int(w) + (int(w) & 1) for w in so.max(axis=1)]
    KDs = [max(w, 2) for w in KDs]
    KSs = [max(w, 2) for w in KSs]
    return core_of, lane_of, q_of, KDs, KSs


def _col_bases(Ks):
    """Column base per q for per-chunk widths Ks; returns (bases[Q], total)."""
    bases = np.zeros(Q, np.int64)
    off = 0
    for c, w in enumerate(Ks):
        for qq in range(QC):
            bases[c * QC + qq] = off + qq * w
        off += QC * w
    return bases, off


def _grid_scatter(core_of, lane_of, q_of, vec_padded):
    out = np.zeros((NC, P, Q), np.float32)
    out[core_of, lane_of, q_of] = vec_padded
    return out


# ---------------- device phase builders ----------------

def _exch_dt():
    return BF16 if EXCH_BF16 else F32


def build_pA(KDs, KSs):
    """indeg via mask row-reduce -> dinv, u; broadcast u to src-major m1."""
    EXT = _exch_dt()
    CD = QC * sum(KDs)
    CS = QC * sum(KSs)
    KDm, KSm = max(KDs), max(KSs)
    nc = bacc.Bacc("TRN2", target_bir_lowering=False, debug=False)
    maskD = nc.dram_tensor("maskD", [P, CD], BF16, kind="ExternalInput")
    xg = nc.dram_tensor("xg", [P, Q], F32, kind="ExternalInput")
    dinv_o = nc.dram_tensor("dinv", [P, Q], F32, kind="ExternalOutput")
    u_o = nc.dram_tensor("u", [P, Q], F32, kind="ExternalOutput")
    m1_o = nc.dram_tensor("m1", [P, CS], EXT, kind="ExternalOutput")
    with tile.TileContext(nc) as tc:
        with tc.tile_pool(name="sb", bufs=1) as pool, \
             tc.tile_pool(name="inp", bufs=3) as inp, \
             tc.tile_pool(name="outp", bufs=3) as outp:
            xg_sb = pool.tile([P, Q], F32, tag="xg")
            nc.sync.dma_start(xg_sb[:], xg.ap())
            indeg = pool.tile([P, Q], F32, tag="indeg")
            dinv_sb = pool.tile([P, Q], F32, tag="dinv")
            u_sb = pool.tile([P, Q], F32, tag="u")
            sq_sb = pool.tile([P, Q], F32, tag="sq")
            ones_sb = pool.tile([P, QC * KSm], EXT, tag="ones")
            nc.vector.memset(ones_sb[:], 1.0)
            offd = offs = 0
            for ci in range(NCH):
                kd, ks = KDs[ci], KSs[ci]
                c0 = ci * QC
                mt = inp.tile([P, QC * KDm], BF16, tag="mchunk")
                nc.sync.dma_start(mt[:, :QC * kd],
                                  maskD.ap()[:, offd:offd + QC * kd])
                nc.vector.tensor_reduce(
                    out=indeg[:, c0:c0 + QC],
                    in_=mt[:, :QC * kd].rearrange("p (q k) -> p q k", k=kd),
                    axis=mybir.AxisListType.X, op=mybir.AluOpType.add)
                nc.scalar.activation(
                    out=sq_sb[:, c0:c0 + QC], in_=indeg[:, c0:c0 + QC],
                    func=mybir.ActivationFunctionType.Sqrt, bias=1.0, scale=1.0)
                nc.vector.reciprocal(out=dinv_sb[:, c0:c0 + QC],
                                     in_=sq_sb[:, c0:c0 + QC])
                nc.vector.tensor_tensor(
                    out=u_sb[:, c0:c0 + QC], in0=xg_sb[:, c0:c0 + QC],
                    in1=dinv_sb[:, c0:c0 + QC], op=mybir.AluOpType.mult)
                bt = outp.tile([P, QC * KSm], EXT, tag="bchunk")
                nc.gpsimd.tensor_tensor(
                    out=bt[:, :QC * ks].rearrange("p (q k) -> p q k", k=ks),
                    in0=ones_sb[:, :QC * ks].rearrange("p (q k) -> p q k", k=ks),
                    in1=u_sb[:, c0:c0 + QC].rearrange(
                        "p (q one) -> p q one", one=1).to_broadcast([P, QC, ks]),
                    op=mybir.AluOpType.mult)
                nc.sync.dma_start(m1_o.ap()[:, offs:offs + QC * ks],
                                  bt[:, :QC * ks])
                offd += QC * kd
                offs += QC * ks
            nc.sync.dma_start(dinv_o.ap(), dinv_sb[:])
            nc.sync.dma_start(u_o.ap(), u_sb[:])
    nc.compile()
    return nc


def build_pB(KDs, KSs):
    """S = segsum(vD1); y = dinv^2 * (S + u); broadcast y to src-major m2."""
    EXT = _exch_dt()
    CD = QC * sum(KDs)
    CS = QC * sum(KSs)
    KDm, KSm = max(KDs), max(KSs)
    nc = bacc.Bacc("TRN2", target_bir_lowering=False, debug=False)
    vD1 = nc.dram_tensor("vD1", [P, CD], EXT, kind="ExternalInput")
    u_i = nc.dram_tensor("u", [P, Q], F32, kind="ExternalInput")
    dinv_i = nc.dram_tensor("dinvg", [P, Q], F32, kind="ExternalInput")
    y_o = nc.dram_tensor("yg", [P, Q], F32, kind="ExternalOutput")
    m2_o = nc.dram_tensor("m2", [P, CS], EXT, kind="ExternalOutput")
    with tile.TileContext(nc) as tc:
        with tc.tile_pool(name="sb", bufs=1) as pool, \
             tc.tile_pool(name="inp", bufs=3) as inp, \
             tc.tile_pool(name="outp", bufs=3) as outp:
            u_sb = pool.tile([P, Q], F32, tag="u")
            dinv_sb = pool.tile([P, Q], F32, tag="dinv")
            d2_sb = pool.tile([P, Q], F32, tag="d2")
            s_sb = pool.tile([P, Q], F32, tag="s")
            y_sb = pool.tile([P, Q], F32, tag="y")
            nc.sync.dma_start(u_sb[:], u_i.ap())
            nc.sync.dma_start(dinv_sb[:], dinv_i.ap())
            nc.vector.tensor_tensor(out=d2_sb[:], in0=dinv_sb[:],
                                    in1=dinv_sb[:], op=mybir.AluOpType.mult)
            ones_sb = pool.tile([P, QC * KSm], EXT, tag="ones")
            nc.vector.memset(ones_sb[:], 1.0)
            offd = offs = 0
            for ci in range(NCH):
                kd, ks = KDs[ci], KSs[ci]
                c0 = ci * QC
                vt = inp.tile([P, QC * KDm], EXT, tag="vchunk")
                nc.sync.dma_start(vt[:, :QC * kd],
                                  vD1.ap()[:, offd:offd + QC * kd])
                nc.vector.tensor_reduce(
                    out=s_sb[:, c0:c0 + QC],
                    in_=vt[:, :QC * kd].rearrange("p (q k) -> p q k", k=kd),
                    axis=mybir.AxisListType.X, op=mybir.AluOpType.add)
                nc.vector.tensor_tensor(
                    out=s_sb[:, c0:c0 + QC], in0=s_sb[:, c0:c0 + QC],
                    in1=u_sb[:, c0:c0 + QC], op=mybir.AluOpType.add)
                nc.vector.tensor_tensor(
                    out=y_sb[:, c0:c0 + QC], in0=s_sb[:, c0:c0 + QC],
                    in1=d2_sb[:, c0:c0 + QC], op=mybir.AluOpType.mult)
                bt = outp.tile([P, QC * KSm], EXT, tag="bchunk")
                nc.gpsimd.tensor_tensor(
                    out=bt[:, :QC * ks].rearrange("p (q k) -> p q k", k=ks),
                    in0=ones_sb[:, :QC * ks].rearrange("p (q k) -> p q k", k=ks),
                    in1=y_sb[:, c0:c0 + QC].rearrange(
                        "p (q one) -> p q one", one=1).to_broadcast([P, QC, ks]),
                    op=mybir.AluOpType.mult)
                nc.sync.dma_start(m2_o.ap()[:, offs:offs + QC * ks],
                                  bt[:, :QC * ks])
                offd += QC * kd
                offs += QC * ks
            nc.sync.dma_start(y_o.ap(), y_sb[:])
    nc.compile()
    return nc


def build_pC(KDs):
    """sp/sm segsums of relu'd y messages; alpha/beta; 16-feature sums."""
    EXT = _exch_dt()
    CD = QC * sum(KDs)
    KDm = max(KDs)
    nc = bacc.Bacc("TRN2", target_bir_lowering=False, debug=False)
    vD2 = nc.dram_tensor("vD2", [P, CD], EXT, kind="ExternalInput")
    dinv_i = nc.dram_tensor("dinvg", [P, Q], F32, kind="ExternalInput")
    y_i = nc.dram_tensor("yg", [P, Q], F32, kind="ExternalInput")
    mask_i = nc.dram_tensor("maskg", [P, Q], F32, kind="ExternalInput")
    cvec = nc.dram_tensor("cvec", [P, 48], F32, kind="ExternalInput")
    acc_o = nc.dram_tensor("acc", [P, 16], F32, kind="ExternalOutput")
    with tile.TileContext(nc) as tc:
        with tc.tile_pool(name="sb", bufs=1) as pool, \
             tc.tile_pool(name="inp", bufs=3) as inp, \
             tc.tile_pool(name="rel", bufs=3) as relp:
            dinv_sb = pool.tile([P, Q], F32, tag="dinv")
            y_sb = pool.tile([P, Q], F32, tag="y")
            mask_sb = pool.tile([P, Q], F32, tag="mask")
            cvec_sb = pool.tile([P, 48], F32, tag="cvec")
            sv_sb = pool.tile([P, Q], F32, tag="sv")
            sp_sb = pool.tile([P, Q], F32, tag="sp")
            for t_sb, t in ((dinv_sb, dinv_i), (y_sb, y_i),
                            (mask_sb, mask_i), (cvec_sb, cvec)):
                nc.sync.dma_start(t_sb[:], t.ap())
            offd = 0
            for ci in range(NCH):
                kd = KDs[ci]
                c0 = ci * QC
                vt = inp.tile([P, QC * KDm], EXT, tag="vchunk")
                nc.sync.dma_start(vt[:, :QC * kd],
                                  vD2.ap()[:, offd:offd + QC * kd])
                nc.vector.tensor_reduce(
                    out=sv_sb[:, c0:c0 + QC],
                    in_=vt[:, :QC * kd].rearrange("p (q k) -> p q k", k=kd),
                    axis=mybir.AxisListType.X, op=mybir.AluOpType.add)
                rt = relp.tile([P, QC * KDm], EXT, tag="rchunk")
                nc.gpsimd.tensor_scalar(
                    out=rt[:, :QC * kd], in0=vt[:, :QC * kd], scalar1=0.0,
                    scalar2=None, op0=mybir.AluOpType.max)
                nc.vector.tensor_reduce(
                    out=sp_sb[:, c0:c0 + QC],
                    in_=rt[:, :QC * kd].rearrange("p (q k) -> p q k", k=kd),
                    axis=mybir.AxisListType.X, op=mybir.AluOpType.add)
                offd += QC * kd
            # node-side terms
            yp = pool.tile([P, Q], F32, tag="yp")
            ym = pool.tile([P, Q], F32, tag="ym")
            alpha = pool.tile([P, Q], F32, tag="alpha")
            beta = pool.tile([P, Q], F32, tag="beta")
            nc.scalar.activation(out=yp[:], in_=y_sb[:],
                                 func=mybir.ActivationFunctionType.Relu)
            nc.vector.tensor_tensor(out=ym[:], in0=yp[:], in1=y_sb[:],
                                    op=mybir.AluOpType.subtract)
            nc.vector.tensor_tensor(out=alpha[:], in0=sp_sb[:], in1=yp[:],
                                    op=mybir.AluOpType.add)
            nc.vector.tensor_tensor(out=alpha[:], in0=alpha[:], in1=dinv_sb[:],
                                    op=mybir.AluOpType.mult)
            nc.vector.tensor_tensor(out=sv_sb[:], in0=sp_sb[:], in1=sv_sb[:],
                                    op=mybir.AluOpType.subtract)  # sm
            nc.vector.tensor_tensor(out=beta[:], in0=sv_sb[:], in1=ym[:],
                                    op=mybir.AluOpType.add)
            nc.vector.tensor_tensor(out=beta[:], in0=beta[:], in1=dinv_sb[:],
                                    op=mybir.AluOpType.mult)
            # 16 features at once in [P, 16, Q] layout
            t1 = pool.tile([P, 16 * Q], F32, tag="t1")
            t2 = pool.tile([P, 16 * Q], F32, tag="t2")
            t13 = t1[:].rearrange("p (j q) -> p j q", j=16)
            t23 = t2[:].rearrange("p (j q) -> p j q", j=16)
            alpha_b = alpha[:].rearrange("p (one q) -> p one q",
                                         one=1).to_broadcast([P, 16, Q])
            beta_b = beta[:].rearrange("p (one q) -> p one q",
                                       one=1).to_broadcast([P, 16, Q])
            mask_b = mask_sb[:].rearrange("p (one q) -> p one q",
                                          one=1).to_broadcast([P, 16, Q])
            a_b = cvec_sb[:, 0:16].rearrange("p (j one) -> p j one",
                                             one=1).to_broadcast([P, 16, Q])
            b_b = cvec_sb[:, 16:32].rearrange("p (j one) -> p j one",
                                              one=1).to_broadcast([P, 16, Q])
            b2_b = cvec_sb[:, 32:48].rearrange("p (j one) -> p j one",
                                               one=1).to_broadcast([P, 16, Q])
            nc.vector.tensor_tensor(out=t13, in0=alpha_b, in1=a_b,
                                    op=mybir.AluOpType.mult)
            nc.gpsimd.tensor_tensor(out=t23, in0=beta_b, in1=b_b,
                                    op=mybir.AluOpType.mult)
            nc.vector.tensor_tensor(out=t13, in0=t13, in1=t23,
                                    op=mybir.AluOpType.add)
            nc.vector.tensor_tensor(out=t13, in0=t13, in1=b2_b,
                                    op=mybir.AluOpType.add)
            nc.scalar.activation(out=t1[:], in_=t1[:],
                                 func=mybir.ActivationFunctionType.Relu)
            nc.vector.tensor_tensor(out=t13, in0=t13, in1=mask_b,
                                    op=mybir.AluOpType.mult)
            acc_sb = pool.tile([P, 16], F32, tag="acc")
            nc.vector.tensor_reduce(out=acc_sb[:], in_=t13,
                                    axis=mybir.AxisListType.X,
                                    op=mybir.AluOpType.add)
            nc.sync.dma_start(acc_o.ap(), acc_sb[:])
    nc.compile()
    return nc


# ---------------- pipeline ----------------

def run_pipeline(inputs, trace=False):
    import ml_dtypes
    exch_np = ml_dtypes.bfloat16 if EXCH_BF16 else np.float32

    x = np.asarray(inputs["x"]).reshape(-1).astype(np.float32)
    ei = np.asarray(inputs["edge_index"])
    src = ei[0].astype(np.int64)
    dst = ei[1].astype(np.int64)
    W1 = np.asarray(inputs["W1"]).astype(np.float64)[0]
    W2 = np.asarray(inputs["W2"]).astype(np.float64)
    b2 = np.asarray(inputs["b2"]).astype(np.float64)
    Wl = np.asarray(inputs["Wl"]).astype(np.float64)
    bl = np.asarray(inputs["bl"]).astype(np.float64)
    a_vec = np.maximum(W1, 0) @ W2
    b_vec = np.maximum(-W1, 0) @ W2

    xpad = np.zeros(NPAD, np.float32)
    xpad[:x.shape[0]] = x
    maskpad = np.zeros(NPAD, np.float32)
    maskpad[:x.shape[0]] = 1.0

    indeg_cnt = np.bincount(dst, minlength=NPAD).astype(np.int64)
    outdeg_cnt = np.bincount(src, minlength=NPAD).astype(np.int64)
    core_of, lane_of, q_of, KDs, KSs = _node_layout(indeg_cnt, outdeg_cnt)
    colD, CD = _col_bases(KDs)
    colS, CS = _col_bases(KSs)

    rank_d = _ranks(dst)
    rank_s = _ranks(src)
    dslot = (core_of[dst] * P + lane_of[dst]) * CD + colD[q_of[dst]] + rank_d
    sslot = (core_of[src] * P + lane_of[src]) * CS + colS[q_of[src]] + rank_s

    x_grids = _grid_scatter(core_of, lane_of, q_of, xpad)
    mask_grids = _grid_scatter(core_of, lane_of, q_of, maskpad)

    maskD = np.zeros(NC * P * CD, np.float32)
    maskD[dslot] = 1.0
    maskD = np.ascontiguousarray(
        maskD.reshape(NC, P, CD).astype(ml_dtypes.bfloat16))

    cvec = np.zeros(48, np.float32)
    cvec[0:16] = a_vec
    cvec[16:32] = b_vec
    cvec[32:48] = b2
    cvec = np.ascontiguousarray(np.tile(cvec, (P, 1)))

    phase_ns = {}

    def run(nc, in_maps, name):
        res = bass_utils.run_bass_kernel_spmd(
            nc, in_maps, core_ids=list(range(NC)), trace=trace)
        phase_ns[name] = res.exec_time_ns
        return res.results

    def exchange(m_stack):
        """Permute per-edge values: src-major layout -> dst-major layout."""
        m_flat = np.ascontiguousarray(m_stack).reshape(-1)
        v = np.zeros(NC * P * CD, exch_np)
        v[dslot] = m_flat[sslot]
        return v.reshape(NC, P, CD)

    ncA = build_pA(KDs, KSs)
    rA = run(ncA, [dict(maskD=maskD[kk], xg=x_grids[kk]) for kk in range(NC)],
             "pA")
    dinv_g = np.stack([rA[kk]["dinv"] for kk in range(NC)])
    u_g = np.stack([rA[kk]["u"] for kk in range(NC)])
    vD1 = exchange(np.stack([rA[kk]["m1"] for kk in range(NC)]))

    ncB = build_pB(KDs, KSs)
    rB = run(ncB, [dict(vD1=vD1[kk], u=u_g[kk], dinvg=dinv_g[kk])
                   for kk in range(NC)], "pB")
    y_g = np.stack([rB[kk]["yg"] for kk in range(NC)])
    vD2 = exchange(np.stack([rB[kk]["m2"] for kk in range(NC)]))

    ncC = build_pC(KDs)
    rC = run(ncC, [dict(vD2=vD2[kk], dinvg=dinv_g[kk], yg=y_g[kk],
                        maskg=mask_grids[kk], cvec=cvec)
                   for kk in range(NC)], "pC")
    acc = np.stack([rC[kk]["acc"] for kk in range(NC)])

    pooled = acc.sum(axis=(0, 1)).astype(np.float64) / float(x.shape[0])
    logits = pooled @ Wl + bl
    m = logits.max()
    out = (logits - m) - np.log(np.exp(logits - m).sum())
    return out[None, :].astype(np.float32), phase_ns


def kernel(**inputs) -> np.ndarray:
    out, _ = run_pipeline(inputs, trace=False)
    return out


# revision 9
# speedup vs baseline: 1.8052x; 1.8052x over previous
"""Trainium2 Bass kernel for nn_Classifier_8461085573484 (2-layer GCN classifier).

Math: with x [N,1] and b1=0 (structurally true for this problem), both GCN
layers collapse to scalar per-node quantities:
  deg_d = indeg(d)+1;  dinv = 1/sqrt(deg);  u = x*dinv
  S_d   = sum_{e->d} u[src];   y = dinv^2*(S + u)   (y = layer1-scalar * dinv)
  sv_d  = sum_{e->d} y[src];  sp_d = sum_{e->d} relu(y[src]);  sm = sp - sv
  alpha = dinv*(sp + relu(y));      beta = dinv*(sm + relu(-y))
  out2  = relu(alpha a^T + beta b^T + b2), a = relu(W1)@W2, b = relu(-W1)@W2
  logits = mean(out2) @ Wl + bl -> log_softmax.

Sharding (8 NeuronCores): nodes are sorted by max(indeg, outdeg) and dealt
round-robin in groups of 1792 to (core, q-chunk); node slot (k, lane, q).
This makes each q-chunk degree-homogeneous, so the padded per-node edge
segments can use a per-chunk width = that chunk's max degree (~1.1x total
slots vs uniform-max padding ~1.7x), and gives every core identical DMA/
compute volume.

Layout: per-edge values live in *expanded row layout* grids [P, cols]:
node (lane, q) owns the KD_c-slot column segment at colD[q]; edge j of that
node (rank by dst or src) occupies slot j, pad slots are zero.  Segment sums
are then plain vector tensor_reduce over [P, QC, KD_c] (no one-hot work
blowup); the neighbor "gather" is a device-side broadcast of the node grid
into the src-major expanded layout (GpSimd engine, overlapping the Vector
reduces).  The host only routes / permutes per-edge values between the
src-major and dst-major layouts (no arithmetic) and applies the O(1)
classifier head.
"""
import contextlib
import ctypes
import sys
import types

import numpy as np

from concourse import bacc, bass, mybir
import concourse.tile as tile
from concourse import bass_utils

P = 128
Q = 98
NSH = P * Q            # 12544 nodes per NC shard
NC = 8
NPAD = NSH * NC        # 100352
N = 100000
F32 = mybir.dt.float32
BF16 = mybir.dt.bfloat16
QC = 14                # q-chunk size (Q = 7 chunks of 14)
NCH = Q // QC          # 7 chunks
G = P * QC             # 1792 nodes per (core, chunk)
EXCH_BF16 = True       # exchange per-edge values in bf16 (halves DMA traffic)


def _install_ntff_shim():
    """Provide antenv.axon_hooks so run_bass_kernel_spmd(trace=True) works."""
    if "antenv.axon_hooks" in sys.modules:
        return
    import antenv

    _hook = None
    try:
        lib = ctypes.CDLL("/opt/axon/libaxon_pjrt.so")
        if hasattr(lib, "axon_start_nrt_profile"):
            lib.axon_start_nrt_profile.argtypes = [
                ctypes.POINTER(ctypes.c_int64), ctypes.c_size_t]
            lib.axon_start_nrt_profile.restype = ctypes.c_int64
            lib.axon_stop_nrt_profile.argtypes = [ctypes.c_char_p]
            lib.axon_stop_nrt_profile.restype = ctypes.c_int64

            @contextlib.contextmanager
            def _hook_impl(output_dir, device_ids):
                import jax
                jax.devices()
                if device_ids:
                    ids = (ctypes.c_int64 * len(device_ids))(*device_ids)
                    rc = lib.axon_start_nrt_profile(ids, len(device_ids))
                else:
                    rc = lib.axon_start_nrt_profile(None, 0)
                if rc != 0:
                    raise RuntimeError(f"axon_start_nrt_profile rc={rc}")
                try:
                    yield
                finally:
                    n = lib.axon_stop_nrt_profile(str(output_dir).encode())
                    if n < 0:
                        raise RuntimeError(f"axon_stop_nrt_profile rc={n}")

            _hook = _hook_impl
    except OSError:
        pass

    mod = types.ModuleType("antenv.axon_hooks")
    mod._hook = _hook
    mod.get_axon_ntff_profile_hook = lambda: mod._hook

    def set_axon_ntff_profile_hook(h):
        mod._hook = h

    mod.set_axon_ntff_profile_hook = set_axon_ntff_profile_hook
    sys.modules["antenv.axon_hooks"] = mod
    antenv.axon_hooks = mod


_install_ntff_shim()


# ---------------- host routing (sharding/layout only, no arithmetic) -------

def _ranks(keys):
    """Rank of each edge within its node group."""
    counts = np.bincount(keys, minlength=NPAD).astype(np.int64)
    starts = np.zeros(NPAD, np.int64)
    starts[1:] = np.cumsum(counts)[:-1]
    order = np.argsort(keys, kind="stable")
    rank = np.empty(keys.shape[0], np.int64)
    rank[order] = np.arange(keys.shape[0], dtype=np.int64) - starts[keys[order]]
    return rank


def _node_layout(indeg, outdeg):
    """Degree-sorted node placement + per-chunk segment widths."""
    key = np.maximum(indeg, outdeg)
    order = np.argsort(-key, kind="stable")        # sorted pos -> node id
    i = np.arange(NPAD)
    g = i // G
    j = i - g * G
    core = g % NC
    chunk = g // NC
    lane = j % P
    q = chunk * QC + (j // P)
    core_of = np.empty(NPAD, np.int64)
    lane_of = np.empty(NPAD, np.int64)
    q_of = np.empty(NPAD, np.int64)
    core_of[order] = core
    lane_of[order] = lane
    q_of[order] = q
    sd = indeg[order].reshape(NCH, NC * G)
    so = outdeg[order].reshape(NCH, NC * G)
    KDs = [int(w) + (int(w) & 1) for w in sd.max(axis=1)]
    KSs = [int(w) + (int(w) & 1) for w in so.max(axis=1)]
    KDs = [max(w, 2) for w in KDs]
    KSs = [max(w, 2) for w in KSs]
    return core_of, lane_of, q_of, KDs, KSs


def _col_bases(Ks):
    """Column base per q for per-chunk widths Ks; returns (bases[Q], total)."""
    bases = np.zeros(Q, np.int64)
    off = 0
    for c, w in enumerate(Ks):
        for qq in range(QC):
            bases[c * QC + qq] = off + qq * w
        off += QC * w
    return bases, off


def _grid_scatter(core_of, lane_of, q_of, vec_padded):
    out = np.zeros((NC, P, Q), np.float32)
    out[core_of, lane_of, q_of] = vec_padded
    return out


# ---------------- device phase builders ----------------

def _exch_dt():
    return BF16 if EXCH_BF16 else F32


def build_pA(KDs, KSs):
    """indeg via mask row-reduce -> dinv, u; broadcast u to src-major m1."""
    EXT = _exch_dt()
    CD = QC * sum(KDs)
    CS = QC * sum(KSs)
    KDm, KSm = max(KDs), max(KSs)
    nc = bacc.Bacc("TRN2", target_bir_lowering=False, debug=False)
    maskD = nc.dram_tensor("maskD", [P, CD], BF16, kind="ExternalInput")
    xg = nc.dram_tensor("xg", [P, Q], F32, kind="ExternalInput")
    dinv_o = nc.dram_tensor("dinv", [P, Q], F32, kind="ExternalOutput")
    u_o = nc.dram_tensor("u", [P, Q], F32, kind="ExternalOutput")
    m1_o = nc.dram_tensor("m1", [P, CS], EXT, kind="ExternalOutput")
    with tile.TileContext(nc) as tc:
        with tc.tile_pool(name="sb", bufs=1) as pool, \
             tc.tile_pool(name="inp", bufs=3) as inp, \
             tc.tile_pool(name="outp", bufs=3) as outp:
            xg_sb = pool.tile([P, Q], F32, tag="xg")
            nc.sync.dma_start(xg_sb[:], xg.ap())
            indeg = pool.tile([P, Q], F32, tag="indeg")
            dinv_sb = pool.tile([P, Q], F32, tag="dinv")
            u_sb = pool.tile([P, Q], F32, tag="u")
            sq_sb = pool.tile([P, Q], F32, tag="sq")
            ones_sb = pool.tile([P, QC * KSm], EXT, tag="ones")
            nc.vector.memset(ones_sb[:], 1.0)
            offd = offs = 0
            for ci in range(NCH):
                kd, ks = KDs[ci], KSs[ci]
                c0 = ci * QC
                mt = inp.tile([P, QC * KDm], BF16, tag="mchunk")
                nc.sync.dma_start(mt[:, :QC * kd],
                                  maskD.ap()[:, offd:offd + QC * kd])
                nc.vector.tensor_reduce(
                    out=indeg[:, c0:c0 + QC],
                    in_=mt[:, :QC * kd].rearrange("p (q k) -> p q k", k=kd),
                    axis=mybir.AxisListType.X, op=mybir.AluOpType.add)
                nc.scalar.activation(
                    out=sq_sb[:, c0:c0 + QC], in_=indeg[:, c0:c0 + QC],
                    func=mybir.ActivationFunctionType.Sqrt, bias=1.0, scale=1.0)
                nc.vector.reciprocal(out=dinv_sb[:, c0:c0 + QC],
                                     in_=sq_sb[:, c0:c0 + QC])
                nc.vector.tensor_tensor(
                    out=u_sb[:, c0:c0 + QC], in0=xg_sb[:, c0:c0 + QC],
                    in1=dinv_sb[:, c0:c0 + QC], op=mybir.AluOpType.mult)
                bt = outp.tile([P, QC * KSm], EXT, tag="bchunk")
                beng = nc.gpsimd if ci % 2 == 0 else nc.vector
                beng.tensor_tensor(
                    out=bt[:, :QC * ks].rearrange("p (q k) -> p q k", k=ks),
                    in0=ones_sb[:, :QC * ks].rearrange("p (q k) -> p q k", k=ks),
                    in1=u_sb[:, c0:c0 + QC].rearrange(
                        "p (q one) -> p q one", one=1).to_broadcast([P, QC, ks]),
                    op=mybir.AluOpType.mult)
                nc.sync.dma_start(m1_o.ap()[:, offs:offs + QC * ks],
                                  bt[:, :QC * ks])
                offd += QC * kd
                offs += QC * ks
            nc.sync.dma_start(dinv_o.ap(), dinv_sb[:])
            nc.sync.dma_start(u_o.ap(), u_sb[:])
    nc.compile()
    return nc


def build_pB(KDs, KSs):
    """S = segsum(vD1); y = dinv^2 * (S + u); broadcast y to src-major m2."""
    EXT = _exch_dt()
    CD = QC * sum(KDs)
    CS = QC * sum(KSs)
    KDm, KSm = max(KDs), max(KSs)
    nc = bacc.Bacc("TRN2", target_bir_lowering=False, debug=False)
    vD1 = nc.dram_tensor("vD1", [P, CD], EXT, kind="ExternalInput")
    u_i = nc.dram_tensor("u", [P, Q], F32, kind="ExternalInput")
    dinv_i = nc.dram_tensor("dinvg", [P, Q], F32, kind="ExternalInput")
    y_o = nc.dram_tensor("yg", [P, Q], F32, kind="ExternalOutput")
    m2_o = nc.dram_tensor("m2", [P, CS], EXT, kind="ExternalOutput")
    with tile.TileContext(nc) as tc:
        with tc.tile_pool(name="sb", bufs=1) as pool, \
             tc.tile_pool(name="inp", bufs=3) as inp, \
             tc.tile_pool(name="outp", bufs=3) as outp:
            u_sb = pool.tile([P, Q], F32, tag="u")
            dinv_sb = pool.tile([P, Q], F32, tag="dinv")
            d2_sb = pool.tile([P, Q], F32, tag="d2")
            s_sb = pool.tile([P, Q], F32, tag="s")
            y_sb = pool.tile([P, Q], F32, tag="y")
            nc.sync.dma_start(u_sb[:], u_i.ap())
            nc.sync.dma_start(dinv_sb[:], dinv_i.ap())
            nc.vector.tensor_tensor(out=d2_sb[:], in0=dinv_sb[:],
                                    in1=dinv_sb[:], op=mybir.AluOpType.mult)
            ones_sb = pool.tile([P, QC * KSm], EXT, tag="ones")
            nc.vector.memset(ones_sb[:], 1.0)
            offd = offs = 0
            for ci in range(NCH):
                kd, ks = KDs[ci], KSs[ci]
                c0 = ci * QC
                vt = inp.tile([P, QC * KDm], EXT, tag="vchunk")
                nc.sync.dma_start(vt[:, :QC * kd],
                                  vD1.ap()[:, offd:offd + QC * kd])
                nc.vector.tensor_reduce(
                    out=s_sb[:, c0:c0 + QC],
                    in_=vt[:, :QC * kd].rearrange("p (q k) -> p q k", k=kd),
                    axis=mybir.AxisListType.X, op=mybir.AluOpType.add)
                nc.vector.tensor_tensor(
                    out=s_sb[:, c0:c0 + QC], in0=s_sb[:, c0:c0 + QC],
                    in1=u_sb[:, c0:c0 + QC], op=mybir.AluOpType.add)
                nc.vector.tensor_tensor(
                    out=y_sb[:, c0:c0 + QC], in0=s_sb[:, c0:c0 + QC],
                    in1=d2_sb[:, c0:c0 + QC], op=mybir.AluOpType.mult)
                bt = outp.tile([P, QC * KSm], EXT, tag="bchunk")
                beng = nc.gpsimd if ci % 2 == 0 else nc.vector
                beng.tensor_tensor(
                    out=bt[:, :QC * ks].rearrange("p (q k) -> p q k", k=ks),
                    in0=ones_sb[:, :QC * ks].rearrange("p (q k) -> p q k", k=ks),
                    in1=y_sb[:, c0:c0 + QC].rearrange(
                        "p (q one) -> p q one", one=1).to_broadcast([P, QC, ks]),
                    op=mybir.AluOpType.mult)
                nc.sync.dma_start(m2_o.ap()[:, offs:offs + QC * ks],
                                  bt[:, :QC * ks])
                offd += QC * kd
                offs += QC * ks
            nc.sync.dma_start(y_o.ap(), y_sb[:])
    nc.compile()
    return nc


def build_pC(KDs):
    """sp/sm segsums of relu'd y messages; alpha/beta; 16-feature sums."""
    EXT = _exch_dt()
    CD = QC * sum(KDs)
    KDm = max(KDs)
    nc = bacc.Bacc("TRN2", target_bir_lowering=False, debug=False)
    vD2 = nc.dram_tensor("vD2", [P, CD], EXT, kind="ExternalInput")
    dinv_i = nc.dram_tensor("dinvg", [P, Q], F32, kind="ExternalInput")
    y_i = nc.dram_tensor("yg", [P, Q], F32, kind="ExternalInput")
    mask_i = nc.dram_tensor("maskg", [P, Q], F32, kind="ExternalInput")
    cvec = nc.dram_tensor("cvec", [P, 48], F32, kind="ExternalInput")
    acc_o = nc.dram_tensor("acc", [P, 16], F32, kind="ExternalOutput")
    with tile.TileContext(nc) as tc:
        with tc.tile_pool(name="sb", bufs=1) as pool, \
             tc.tile_pool(name="inp", bufs=3) as inp, \
             tc.tile_pool(name="rel", bufs=3) as relp:
            dinv_sb = pool.tile([P, Q], F32, tag="dinv")
            y_sb = pool.tile([P, Q], F32, tag="y")
            mask_sb = pool.tile([P, Q], F32, tag="mask")
            cvec_sb = pool.tile([P, 48], F32, tag="cvec")
            sv_sb = pool.tile([P, Q], F32, tag="sv")
            sp_sb = pool.tile([P, Q], F32, tag="sp")
            for t_sb, t in ((dinv_sb, dinv_i), (y_sb, y_i),
                            (mask_sb, mask_i), (cvec_sb, cvec)):
                nc.sync.dma_start(t_sb[:], t.ap())
            offd = 0
            for ci in range(NCH):
                kd = KDs[ci]
                c0 = ci * QC
                vt = inp.tile([P, QC * KDm], EXT, tag="vchunk")
                nc.sync.dma_start(vt[:, :QC * kd],
                                  vD2.ap()[:, offd:offd + QC * kd])
                nc.vector.tensor_reduce(
                    out=sv_sb[:, c0:c0 + QC],
                    in_=vt[:, :QC * kd].rearrange("p (q k) -> p q k", k=kd),
                    axis=mybir.AxisListType.X, op=mybir.AluOpType.add)
                rt = relp.tile([P, QC * KDm], EXT, tag="rchunk")
                nc.scalar.activation(out=rt[:, :QC * kd], in_=vt[:, :QC * kd],
                                     func=mybir.ActivationFunctionType.Relu)
                nc.vector.tensor_reduce(
                    out=sp_sb[:, c0:c0 + QC],
                    in_=rt[:, :QC * kd].rearrange("p (q k) -> p q k", k=kd),
                    axis=mybir.AxisListType.X, op=mybir.AluOpType.add)
                offd += QC * kd
            # node-side terms
            yp = pool.tile([P, Q], F32, tag="yp")
            ym = pool.tile([P, Q], F32, tag="ym")
            alpha = pool.tile([P, Q], F32, tag="alpha")
            beta = pool.tile([P, Q], F32, tag="beta")
            nc.scalar.activation(out=yp[:], in_=y_sb[:],
                                 func=mybir.ActivationFunctionType.Relu)
            nc.vector.tensor_tensor(out=ym[:], in0=yp[:], in1=y_sb[:],
                                    op=mybir.AluOpType.subtract)
            nc.vector.tensor_tensor(out=alpha[:], in0=sp_sb[:], in1=yp[:],
                                    op=mybir.AluOpType.add)
            nc.vector.tensor_tensor(out=alpha[:], in0=alpha[:], in1=dinv_sb[:],
                                    op=mybir.AluOpType.mult)
            nc.vector.tensor_tensor(out=sv_sb[:], in0=sp_sb[:], in1=sv_sb[:],
                                    op=mybir.AluOpType.subtract)  # sm
            nc.vector.tensor_tensor(out=beta[:], in0=sv_sb[:], in1=ym[:],
                                    op=mybir.AluOpType.add)
            nc.vector.tensor_tensor(out=beta[:], in0=beta[:], in1=dinv_sb[:],
                                    op=mybir.AluOpType.mult)
            # 16 features at once in [P, 16, Q] layout
            t1 = pool.tile([P, 16 * Q], F32, tag="t1")
            t2 = pool.tile([P, 16 * Q], F32, tag="t2")
            t13 = t1[:].rearrange("p (j q) -> p j q", j=16)
            t23 = t2[:].rearrange("p (j q) -> p j q", j=16)
            alpha_b = alpha[:].rearrange("p (one q) -> p one q",
                                         one=1).to_broadcast([P, 16, Q])
            beta_b = beta[:].rearrange("p (one q) -> p one q",
                                       one=1).to_broadcast([P, 16, Q])
            mask_b = mask_sb[:].rearrange("p (one q) -> p one q",
                                          one=1).to_broadcast([P, 16, Q])
            a_b = cvec_sb[:, 0:16].rearrange("p (j one) -> p j one",
                                             one=1).to_broadcast([P, 16, Q])
            b_b = cvec_sb[:, 16:32].rearrange("p (j one) -> p j one",
                                              one=1).to_broadcast([P, 16, Q])
            b2_b = cvec_sb[:, 32:48].rearrange("p (j one) -> p j one",
                                               one=1).to_broadcast([P, 16, Q])
            nc.vector.tensor_tensor(out=t13, in0=alpha_b, in1=a_b,
                                    op=mybir.AluOpType.mult)
            nc.gpsimd.tensor_tensor(out=t23, in0=beta_b, in1=b_b,
                                    op=mybir.AluOpType.mult)
            nc.vector.tensor_tensor(out=t13, in0=t13, in1=t23,
                                    op=mybir.AluOpType.add)
            nc.vector.tensor_tensor(out=t13, in0=t13, in1=b2_b,
                                    op=mybir.AluOpType.add)
            nc.scalar.activation(out=t1[:], in_=t1[:],
                                 func=mybir.ActivationFunctionType.Relu)
            nc.vector.tensor_tensor(out=t13, in0=t13, in1=mask_b,
                                    op=mybir.AluOpType.mult)
            acc_sb = pool.tile([P, 16], F32, tag="acc")
            nc.vector.tensor_reduce(out=acc_sb[:], in_=t13,
                                    axis=mybir.AxisListType.X,
                                    op=mybir.AluOpType.add)
            nc.sync.dma_start(acc_o.ap(), acc_sb[:])
    nc.compile()
    return nc


# ---------------- pipeline ----------------

def run_pipeline(inputs, trace=False):
    import ml_dtypes
    exch_np = ml_dtypes.bfloat16 if EXCH_BF16 else np.float32

    x = np.asarray(inputs["x"]).reshape(-1).astype(np.float32)
    ei = np.asarray(inputs["edge_index"])
    src = ei[0].astype(np.int64)
    dst = ei[1].astype(np.int64)
    W1 = np.asarray(inputs["W1"]).astype(np.float64)[0]
    W2 = np.asarray(inputs["W2"]).astype(np.float64)
    b2 = np.asarray(inputs["b2"]).astype(np.float64)
    Wl = np.asarray(inputs["Wl"]).astype(np.float64)
    bl = np.asarray(inputs["bl"]).astype(np.float64)
    a_vec = np.maximum(W1, 0) @ W2
    b_vec = np.maximum(-W1, 0) @ W2

    xpad = np.zeros(NPAD, np.float32)
    xpad[:x.shape[0]] = x
    maskpad = np.zeros(NPAD, np.float32)
    maskpad[:x.shape[0]] = 1.0

    indeg_cnt = np.bincount(dst, minlength=NPAD).astype(np.int64)
    outdeg_cnt = np.bincount(src, minlength=NPAD).astype(np.int64)
    core_of, lane_of, q_of, KDs, KSs = _node_layout(indeg_cnt, outdeg_cnt)
    colD, CD = _col_bases(KDs)
    colS, CS = _col_bases(KSs)

    rank_d = _ranks(dst)
    rank_s = _ranks(src)
    dslot = (core_of[dst] * P + lane_of[dst]) * CD + colD[q_of[dst]] + rank_d
    sslot = (core_of[src] * P + lane_of[src]) * CS + colS[q_of[src]] + rank_s

    x_grids = _grid_scatter(core_of, lane_of, q_of, xpad)
    mask_grids = _grid_scatter(core_of, lane_of, q_of, maskpad)

    maskD = np.zeros(NC * P * CD, np.float32)
    maskD[dslot] = 1.0
    maskD = np.ascontiguousarray(
        maskD.reshape(NC, P, CD).astype(ml_dtypes.bfloat16))

    cvec = np.zeros(48, np.float32)
    cvec[0:16] = a_vec
    cvec[16:32] = b_vec
    cvec[32:48] = b2
    cvec = np.ascontiguousarray(np.tile(cvec, (P, 1)))

    phase_ns = {}

    def run(nc, in_maps, name):
        res = bass_utils.run_bass_kernel_spmd(
            nc, in_maps, core_ids=list(range(NC)), trace=trace)
        phase_ns[name] = res.exec_time_ns
        return res.results

    def exchange(m_stack):
        """Permute per-edge values: src-major layout -> dst-major layout."""
        m_flat = np.ascontiguousarray(m_stack).reshape(-1)
        v = np.zeros(NC * P * CD, exch_np)
        v[dslot] = m_flat[sslot]
        return v.reshape(NC, P, CD)

    ncA = build_pA(KDs, KSs)
    rA = run(ncA, [dict(maskD=maskD[kk], xg=x_grids[kk]) for kk in range(NC)],
             "pA")
    dinv_g = np.stack([rA[kk]["dinv"] for kk in range(NC)])
    u_g = np.stack([rA[kk]["u"] for kk in range(NC)])
    vD1 = exchange(np.stack([rA[kk]["m1"] for kk in range(NC)]))

    ncB = build_pB(KDs, KSs)
    rB = run(ncB, [dict(vD1=vD1[kk], u=u_g[kk], dinvg=dinv_g[kk])
                   for kk in range(NC)], "pB")
    y_g = np.stack([rB[kk]["yg"] for kk in range(NC)])
    vD2 = exchange(np.stack([rB[kk]["m2"] for kk in range(NC)]))

    ncC = build_pC(KDs)
    rC = run(ncC, [dict(vD2=vD2[kk], dinvg=dinv_g[kk], yg=y_g[kk],
                        maskg=mask_grids[kk], cvec=cvec)
                   for kk in range(NC)], "pC")
    acc = np.stack([rC[kk]["acc"] for kk in range(NC)])

    pooled = acc.sum(axis=(0, 1)).astype(np.float64) / float(x.shape[0])
    logits = pooled @ Wl + bl
    m = logits.max()
    out = (logits - m) - np.log(np.exp(logits - m).sum())
    return out[None, :].astype(np.float32), phase_ns


def kernel(**inputs) -> np.ndarray:
    out, _ = run_pipeline(inputs, trace=False)
    return out


# revision 10
# speedup vs baseline: 1.8120x; 1.0038x over previous
"""Trainium2 Bass kernel for nn_Classifier_8461085573484 (2-layer GCN classifier).

Math: with x [N,1] and b1=0 (structurally true for this problem), both GCN
layers collapse to scalar per-node quantities:
  deg_d = indeg(d)+1;  dinv = 1/sqrt(deg);  u = x*dinv
  S_d   = sum_{e->d} u[src];   y = dinv^2*(S + u)   (y = layer1-scalar * dinv)
  sv_d  = sum_{e->d} y[src];  sp_d = sum_{e->d} relu(y[src]);  sm = sp - sv
  alpha = dinv*(sp + relu(y));      beta = dinv*(sm + relu(-y))
  out2  = relu(alpha a^T + beta b^T + b2), a = relu(W1)@W2, b = relu(-W1)@W2
  logits = mean(out2) @ Wl + bl -> log_softmax.

Sharding (8 NeuronCores): nodes are sorted by max(indeg, outdeg) and dealt
round-robin in groups of 1792 to (core, q-chunk); node slot (k, lane, q).
This makes each q-chunk degree-homogeneous, so the padded per-node edge
segments can use a per-chunk width = that chunk's max degree (~1.1x total
slots vs uniform-max padding ~1.7x), and gives every core identical DMA/
compute volume.

Layout: per-edge values live in *expanded row layout* grids [P, cols]:
node (lane, q) owns the KD_c-slot column segment at colD[q]; edge j of that
node (rank by dst or src) occupies slot j, pad slots are zero.  Segment sums
are then plain vector tensor_reduce over [P, QC, KD_c] (no one-hot work
blowup); the neighbor "gather" is a device-side broadcast of the node grid
into the src-major expanded layout (GpSimd engine, overlapping the Vector
reduces).  The host only routes / permutes per-edge values between the
src-major and dst-major layouts (no arithmetic) and applies the O(1)
classifier head.
"""
import contextlib
import ctypes
import sys
import types

import numpy as np

from concourse import bacc, bass, mybir
import concourse.tile as tile
from concourse import bass_utils

P = 128
Q = 98
NSH = P * Q            # 12544 nodes per NC shard
NC = 8
NPAD = NSH * NC        # 100352
N = 100000
F32 = mybir.dt.float32
BF16 = mybir.dt.bfloat16
QC = 14                # q-chunk size (Q = 7 chunks of 14)
NCH = Q // QC          # 7 chunks
G = P * QC             # 1792 nodes per (core, chunk)
EXCH_BF16 = True       # exchange per-edge values in bf16 (halves DMA traffic)


def _install_ntff_shim():
    """Provide antenv.axon_hooks so run_bass_kernel_spmd(trace=True) works."""
    if "antenv.axon_hooks" in sys.modules:
        return
    import antenv

    _hook = None
    try:
        lib = ctypes.CDLL("/opt/axon/libaxon_pjrt.so")
        if hasattr(lib, "axon_start_nrt_profile"):
            lib.axon_start_nrt_profile.argtypes = [
                ctypes.POINTER(ctypes.c_int64), ctypes.c_size_t]
            lib.axon_start_nrt_profile.restype = ctypes.c_int64
            lib.axon_stop_nrt_profile.argtypes = [ctypes.c_char_p]
            lib.axon_stop_nrt_profile.restype = ctypes.c_int64

            @contextlib.contextmanager
            def _hook_impl(output_dir, device_ids):
                import jax
                jax.devices()
                if device_ids:
                    ids = (ctypes.c_int64 * len(device_ids))(*device_ids)
                    rc = lib.axon_start_nrt_profile(ids, len(device_ids))
                else:
                    rc = lib.axon_start_nrt_profile(None, 0)
                if rc != 0:
                    raise RuntimeError(f"axon_start_nrt_profile rc={rc}")
                try:
                    yield
                finally:
                    n = lib.axon_stop_nrt_profile(str(output_dir).encode())
                    if n < 0:
                        raise RuntimeError(f"axon_stop_nrt_profile rc={n}")

            _hook = _hook_impl
    except OSError:
        pass

    mod = types.ModuleType("antenv.axon_hooks")
    mod._hook = _hook
    mod.get_axon_ntff_profile_hook = lambda: mod._hook

    def set_axon_ntff_profile_hook(h):
        mod._hook = h

    mod.set_axon_ntff_profile_hook = set_axon_ntff_profile_hook
    sys.modules["antenv.axon_hooks"] = mod
    antenv.axon_hooks = mod


_install_ntff_shim()


# ---------------- host routing (sharding/layout only, no arithmetic) -------

def _ranks(keys):
    """Rank of each edge within its node group."""
    counts = np.bincount(keys, minlength=NPAD).astype(np.int64)
    starts = np.zeros(NPAD, np.int64)
    starts[1:] = np.cumsum(counts)[:-1]
    order = np.argsort(keys, kind="stable")
    rank = np.empty(keys.shape[0], np.int64)
    rank[order] = np.arange(keys.shape[0], dtype=np.int64) - starts[keys[order]]
    return rank


def _node_layout(indeg, outdeg):
    """Degree-sorted node placement + per-chunk segment widths."""
    key = np.maximum(indeg, outdeg)
    order = np.argsort(-key, kind="stable")        # sorted pos -> node id
    i = np.arange(NPAD)
    g = i // G
    j = i - g * G
    core = g % NC
    chunk = g // NC
    lane = j % P
    q = chunk * QC + (j // P)
    core_of = np.empty(NPAD, np.int64)
    lane_of = np.empty(NPAD, np.int64)
    q_of = np.empty(NPAD, np.int64)
    core_of[order] = core
    lane_of[order] = lane
    q_of[order] = q
    sd = indeg[order].reshape(NCH, NC * G)
    so = outdeg[order].reshape(NCH, NC * G)
    KDs = [int(w) + (int(w) & 1) for w in sd.max(axis=1)]
    KSs = [int(w) + (int(w) & 1) for w in so.max(axis=1)]
    KDs = [max(w, 2) for w in KDs]
    KSs = [max(w, 2) for w in KSs]
    return core_of, lane_of, q_of, KDs, KSs


def _col_bases(Ks):
    """Column base per q for per-chunk widths Ks; returns (bases[Q], total)."""
    bases = np.zeros(Q, np.int64)
    off = 0
    for c, w in enumerate(Ks):
        for qq in range(QC):
            bases[c * QC + qq] = off + qq * w
        off += QC * w
    return bases, off


def _grid_scatter(core_of, lane_of, q_of, vec_padded):
    out = np.zeros((NC, P, Q), np.float32)
    out[core_of, lane_of, q_of] = vec_padded
    return out


# ---------------- device phase builders ----------------

def _exch_dt():
    return BF16 if EXCH_BF16 else F32


def build_pA(KDs, KSs):
    """indeg via mask row-reduce -> dinv, u; broadcast u to src-major m1."""
    EXT = _exch_dt()
    CD = QC * sum(KDs)
    CS = QC * sum(KSs)
    KDm, KSm = max(KDs), max(KSs)
    nc = bacc.Bacc("TRN2", target_bir_lowering=False, debug=False)
    maskD = nc.dram_tensor("maskD", [P, CD], BF16, kind="ExternalInput")
    xg = nc.dram_tensor("xg", [P, Q], F32, kind="ExternalInput")
    dinv_o = nc.dram_tensor("dinv", [P, Q], F32, kind="ExternalOutput")
    u_o = nc.dram_tensor("u", [P, Q], F32, kind="ExternalOutput")
    m1_o = nc.dram_tensor("m1", [P, CS], EXT, kind="ExternalOutput")
    with tile.TileContext(nc) as tc:
        with tc.tile_pool(name="sb", bufs=1) as pool, \
             tc.tile_pool(name="inp", bufs=3) as inp, \
             tc.tile_pool(name="outp", bufs=3) as outp:
            xg_sb = pool.tile([P, Q], F32, tag="xg")
            nc.sync.dma_start(xg_sb[:], xg.ap())
            indeg = pool.tile([P, Q], BF16, tag="indeg")
            dinv_sb = pool.tile([P, Q], F32, tag="dinv")
            u_sb = pool.tile([P, Q], F32, tag="u")
            sq_sb = pool.tile([P, Q], F32, tag="sq")
            ones_sb = pool.tile([P, QC * KSm], EXT, tag="ones")
            nc.vector.memset(ones_sb[:], 1.0)
            offd = offs = 0
            for ci in range(NCH):
                kd, ks = KDs[ci], KSs[ci]
                c0 = ci * QC
                mt = inp.tile([P, QC * KDm], BF16, tag="mchunk")
                nc.sync.dma_start(mt[:, :QC * kd],
                                  maskD.ap()[:, offd:offd + QC * kd])
                with nc.allow_low_precision("integer counts exact in bf16"):
                    nc.vector.tensor_reduce(
                        out=indeg[:, c0:c0 + QC],
                        in_=mt[:, :QC * kd].rearrange("p (q k) -> p q k", k=kd),
                        axis=mybir.AxisListType.X, op=mybir.AluOpType.add)
                nc.scalar.activation(
                    out=sq_sb[:, c0:c0 + QC], in_=indeg[:, c0:c0 + QC],
                    func=mybir.ActivationFunctionType.Sqrt, bias=1.0, scale=1.0)
                nc.vector.reciprocal(out=dinv_sb[:, c0:c0 + QC],
                                     in_=sq_sb[:, c0:c0 + QC])
                nc.vector.tensor_tensor(
                    out=u_sb[:, c0:c0 + QC], in0=xg_sb[:, c0:c0 + QC],
                    in1=dinv_sb[:, c0:c0 + QC], op=mybir.AluOpType.mult)
                bt = outp.tile([P, QC * KSm], EXT, tag="bchunk")
                beng = nc.gpsimd if ci % 2 == 0 else nc.vector
                beng.tensor_tensor(
                    out=bt[:, :QC * ks].rearrange("p (q k) -> p q k", k=ks),
                    in0=ones_sb[:, :QC * ks].rearrange("p (q k) -> p q k", k=ks),
                    in1=u_sb[:, c0:c0 + QC].rearrange(
                        "p (q one) -> p q one", one=1).to_broadcast([P, QC, ks]),
                    op=mybir.AluOpType.mult)
                nc.scalar.dma_start(m1_o.ap()[:, offs:offs + QC * ks],
                                    bt[:, :QC * ks])
                offd += QC * kd
                offs += QC * ks
            nc.scalar.dma_start(dinv_o.ap(), dinv_sb[:])
            nc.scalar.dma_start(u_o.ap(), u_sb[:])
    nc.compile()
    return nc


def build_pB(KDs, KSs):
    """S = segsum(vD1); y = dinv^2 * (S + u); broadcast y to src-major m2."""
    EXT = _exch_dt()
    CD = QC * sum(KDs)
    CS = QC * sum(KSs)
    KDm, KSm = max(KDs), max(KSs)
    nc = bacc.Bacc("TRN2", target_bir_lowering=False, debug=False)
    vD1 = nc.dram_tensor("vD1", [P, CD], EXT, kind="ExternalInput")
    u_i = nc.dram_tensor("u", [P, Q], F32, kind="ExternalInput")
    dinv_i = nc.dram_tensor("dinvg", [P, Q], F32, kind="ExternalInput")
    y_o = nc.dram_tensor("yg", [P, Q], F32, kind="ExternalOutput")
    m2_o = nc.dram_tensor("m2", [P, CS], EXT, kind="ExternalOutput")
    with tile.TileContext(nc) as tc:
        with tc.tile_pool(name="sb", bufs=1) as pool, \
             tc.tile_pool(name="inp", bufs=3) as inp, \
             tc.tile_pool(name="outp", bufs=3) as outp:
            u_sb = pool.tile([P, Q], F32, tag="u")
            dinv_sb = pool.tile([P, Q], F32, tag="dinv")
            d2_sb = pool.tile([P, Q], F32, tag="d2")
            s_sb = pool.tile([P, Q], BF16, tag="s")
            y_sb = pool.tile([P, Q], F32, tag="y")
            nc.sync.dma_start(u_sb[:], u_i.ap())
            nc.sync.dma_start(dinv_sb[:], dinv_i.ap())
            nc.vector.tensor_tensor(out=d2_sb[:], in0=dinv_sb[:],
                                    in1=dinv_sb[:], op=mybir.AluOpType.mult)
            ones_sb = pool.tile([P, QC * KSm], EXT, tag="ones")
            nc.vector.memset(ones_sb[:], 1.0)
            offd = offs = 0
            for ci in range(NCH):
                kd, ks = KDs[ci], KSs[ci]
                c0 = ci * QC
                vt = inp.tile([P, QC * KDm], EXT, tag="vchunk")
                nc.sync.dma_start(vt[:, :QC * kd],
                                  vD1.ap()[:, offd:offd + QC * kd])
                with nc.allow_low_precision("segment sums tolerate bf16"):
                    nc.vector.tensor_reduce(
                        out=s_sb[:, c0:c0 + QC],
                        in_=vt[:, :QC * kd].rearrange("p (q k) -> p q k", k=kd),
                        axis=mybir.AxisListType.X, op=mybir.AluOpType.add)
                nc.vector.tensor_tensor(
                    out=y_sb[:, c0:c0 + QC], in0=s_sb[:, c0:c0 + QC],
                    in1=u_sb[:, c0:c0 + QC], op=mybir.AluOpType.add)
                nc.vector.tensor_tensor(
                    out=y_sb[:, c0:c0 + QC], in0=y_sb[:, c0:c0 + QC],
                    in1=d2_sb[:, c0:c0 + QC], op=mybir.AluOpType.mult)
                bt = outp.tile([P, QC * KSm], EXT, tag="bchunk")
                beng = nc.gpsimd if ci % 2 == 0 else nc.vector
                beng.tensor_tensor(
                    out=bt[:, :QC * ks].rearrange("p (q k) -> p q k", k=ks),
                    in0=ones_sb[:, :QC * ks].rearrange("p (q k) -> p q k", k=ks),
                    in1=y_sb[:, c0:c0 + QC].rearrange(
                        "p (q one) -> p q one", one=1).to_broadcast([P, QC, ks]),
                    op=mybir.AluOpType.mult)
                nc.scalar.dma_start(m2_o.ap()[:, offs:offs + QC * ks],
                                    bt[:, :QC * ks])
                offd += QC * kd
                offs += QC * ks
            nc.scalar.dma_start(y_o.ap(), y_sb[:])
    nc.compile()
    return nc


def build_pC(KDs, b2_zero):
    """sp/sm segsums of relu'd y messages; alpha/beta; 16-feature sums."""
    EXT = _exch_dt()
    CD = QC * sum(KDs)
    KDm = max(KDs)
    nc = bacc.Bacc("TRN2", target_bir_lowering=False, debug=False)
    vD2 = nc.dram_tensor("vD2", [P, CD], EXT, kind="ExternalInput")
    dinv_i = nc.dram_tensor("dinvg", [P, Q], F32, kind="ExternalInput")
    y_i = nc.dram_tensor("yg", [P, Q], F32, kind="ExternalInput")
    mask_i = nc.dram_tensor("maskg", [P, Q], F32, kind="ExternalInput")
    cvec = nc.dram_tensor("cvec", [P, 48], F32, kind="ExternalInput")
    acc_o = nc.dram_tensor("acc", [P, 16], F32, kind="ExternalOutput")
    with tile.TileContext(nc) as tc:
        with tc.tile_pool(name="sb", bufs=1) as pool, \
             tc.tile_pool(name="inp", bufs=3) as inp, \
             tc.tile_pool(name="rel", bufs=3) as relp:
            dinv_sb = pool.tile([P, Q], F32, tag="dinv")
            y_sb = pool.tile([P, Q], F32, tag="y")
            mask_sb = pool.tile([P, Q], F32, tag="mask")
            cvec_sb = pool.tile([P, 48], F32, tag="cvec")
            sv_sb = pool.tile([P, Q], BF16, tag="sv")
            sp_sb = pool.tile([P, Q], BF16, tag="sp")
            for t_sb, t in ((dinv_sb, dinv_i), (y_sb, y_i),
                            (mask_sb, mask_i), (cvec_sb, cvec)):
                nc.sync.dma_start(t_sb[:], t.ap())
            offd = 0
            for ci in range(NCH):
                kd = KDs[ci]
                c0 = ci * QC
                vt = inp.tile([P, QC * KDm], EXT, tag="vchunk")
                nc.sync.dma_start(vt[:, :QC * kd],
                                  vD2.ap()[:, offd:offd + QC * kd])
                with nc.allow_low_precision("segment sums tolerate bf16"):
                    nc.vector.tensor_reduce(
                        out=sv_sb[:, c0:c0 + QC],
                        in_=vt[:, :QC * kd].rearrange("p (q k) -> p q k", k=kd),
                        axis=mybir.AxisListType.X, op=mybir.AluOpType.add)
                rt = relp.tile([P, QC * KDm], EXT, tag="rchunk")
                nc.scalar.activation(out=rt[:, :QC * kd], in_=vt[:, :QC * kd],
                                     func=mybir.ActivationFunctionType.Relu)
                with nc.allow_low_precision("segment sums tolerate bf16"):
                    nc.vector.tensor_reduce(
                        out=sp_sb[:, c0:c0 + QC],
                        in_=rt[:, :QC * kd].rearrange("p (q k) -> p q k", k=kd),
                        axis=mybir.AxisListType.X, op=mybir.AluOpType.add)
                offd += QC * kd
            # node-side terms; mask folded into the dinv multiplier (valid
            # because pad nodes then contribute relu(b2)=0 when b2 == 0;
            # the b2 != 0 case keeps the explicit mask path below)
            yp = pool.tile([P, Q], F32, tag="yp")
            ym = pool.tile([P, Q], F32, tag="ym")
            alpha = pool.tile([P, Q], F32, tag="alpha")
            beta = pool.tile([P, Q], F32, tag="beta")
            dm = pool.tile([P, Q], F32, tag="dm")
            if b2_zero:
                nc.vector.tensor_tensor(out=dm[:], in0=dinv_sb[:],
                                        in1=mask_sb[:], op=mybir.AluOpType.mult)
            else:
                nc.scalar.activation(out=dm[:], in_=dinv_sb[:],
                                     func=mybir.ActivationFunctionType.Copy)
            nc.scalar.activation(out=yp[:], in_=y_sb[:],
                                 func=mybir.ActivationFunctionType.Relu)
            nc.vector.tensor_tensor(out=ym[:], in0=yp[:], in1=y_sb[:],
                                    op=mybir.AluOpType.subtract)
            nc.vector.tensor_tensor(out=alpha[:], in0=sp_sb[:], in1=yp[:],
                                    op=mybir.AluOpType.add)
            nc.vector.tensor_tensor(out=alpha[:], in0=alpha[:], in1=dm[:],
                                    op=mybir.AluOpType.mult)
            nc.vector.tensor_tensor(out=sv_sb[:], in0=sp_sb[:], in1=sv_sb[:],
                                    op=mybir.AluOpType.subtract)  # sm
            nc.vector.tensor_tensor(out=beta[:], in0=sv_sb[:], in1=ym[:],
                                    op=mybir.AluOpType.add)
            nc.vector.tensor_tensor(out=beta[:], in0=beta[:], in1=dm[:],
                                    op=mybir.AluOpType.mult)
            # 16 features at once in [P, 16, Q] layout
            t1 = pool.tile([P, 16 * Q], F32, tag="t1")
            t2 = pool.tile([P, 16 * Q], F32, tag="t2")
            t13 = t1[:].rearrange("p (j q) -> p j q", j=16)
            t23 = t2[:].rearrange("p (j q) -> p j q", j=16)
            alpha_b = alpha[:].rearrange("p (one q) -> p one q",
                                         one=1).to_broadcast([P, 16, Q])
            beta_b = beta[:].rearrange("p (one q) -> p one q",
                                       one=1).to_broadcast([P, 16, Q])
            mask_b = mask_sb[:].rearrange("p (one q) -> p one q",
                                          one=1).to_broadcast([P, 16, Q])
            a_b = cvec_sb[:, 0:16].rearrange("p (j one) -> p j one",
                                             one=1).to_broadcast([P, 16, Q])
            b_b = cvec_sb[:, 16:32].rearrange("p (j one) -> p j one",
                                              one=1).to_broadcast([P, 16, Q])
            b2_b = cvec_sb[:, 32:48].rearrange("p (j one) -> p j one",
                                               one=1).to_broadcast([P, 16, Q])
            nc.vector.tensor_tensor(out=t13, in0=alpha_b, in1=a_b,
                                    op=mybir.AluOpType.mult)
            nc.gpsimd.tensor_tensor(out=t23, in0=beta_b, in1=b_b,
                                    op=mybir.AluOpType.mult)
            nc.vector.tensor_tensor(out=t13, in0=t13, in1=t23,
                                    op=mybir.AluOpType.add)
            if not b2_zero:
                nc.vector.tensor_tensor(out=t13, in0=t13, in1=b2_b,
                                        op=mybir.AluOpType.add)
            nc.scalar.activation(out=t1[:], in_=t1[:],
                                 func=mybir.ActivationFunctionType.Relu)
            if not b2_zero:
                nc.vector.tensor_tensor(out=t13, in0=t13, in1=mask_b,
                                        op=mybir.AluOpType.mult)
            acc_sb = pool.tile([P, 16], F32, tag="acc")
            nc.vector.tensor_reduce(out=acc_sb[:], in_=t13,
                                    axis=mybir.AxisListType.X,
                                    op=mybir.AluOpType.add)
            nc.scalar.dma_start(acc_o.ap(), acc_sb[:])
    nc.compile()
    return nc


# ---------------- pipeline ----------------

def run_pipeline(inputs, trace=False):
    import ml_dtypes
    exch_np = ml_dtypes.bfloat16 if EXCH_BF16 else np.float32

    x = np.asarray(inputs["x"]).reshape(-1).astype(np.float32)
    ei = np.asarray(inputs["edge_index"])
    src = ei[0].astype(np.int64)
    dst = ei[1].astype(np.int64)
    W1 = np.asarray(inputs["W1"]).astype(np.float64)[0]
    W2 = np.asarray(inputs["W2"]).astype(np.float64)
    b2 = np.asarray(inputs["b2"]).astype(np.float64)
    Wl = np.asarray(inputs["Wl"]).astype(np.float64)
    bl = np.asarray(inputs["bl"]).astype(np.float64)
    a_vec = np.maximum(W1, 0) @ W2
    b_vec = np.maximum(-W1, 0) @ W2

    xpad = np.zeros(NPAD, np.float32)
    xpad[:x.shape[0]] = x
    maskpad = np.zeros(NPAD, np.float32)
    maskpad[:x.shape[0]] = 1.0

    indeg_cnt = np.bincount(dst, minlength=NPAD).astype(np.int64)
    outdeg_cnt = np.bincount(src, minlength=NPAD).astype(np.int64)
    core_of, lane_of, q_of, KDs, KSs = _node_layout(indeg_cnt, outdeg_cnt)
    colD, CD = _col_bases(KDs)
    colS, CS = _col_bases(KSs)

    rank_d = _ranks(dst)
    rank_s = _ranks(src)
    dslot = (core_of[dst] * P + lane_of[dst]) * CD + colD[q_of[dst]] + rank_d
    sslot = (core_of[src] * P + lane_of[src]) * CS + colS[q_of[src]] + rank_s

    x_grids = _grid_scatter(core_of, lane_of, q_of, xpad)
    mask_grids = _grid_scatter(core_of, lane_of, q_of, maskpad)

    maskD = np.zeros(NC * P * CD, np.float32)
    maskD[dslot] = 1.0
    maskD = np.ascontiguousarray(
        maskD.reshape(NC, P, CD).astype(ml_dtypes.bfloat16))

    cvec = np.zeros(48, np.float32)
    cvec[0:16] = a_vec
    cvec[16:32] = b_vec
    cvec[32:48] = b2
    cvec = np.ascontiguousarray(np.tile(cvec, (P, 1)))

    phase_ns = {}

    def run(nc, in_maps, name):
        res = bass_utils.run_bass_kernel_spmd(
            nc, in_maps, core_ids=list(range(NC)), trace=trace)
        phase_ns[name] = res.exec_time_ns
        return res.results

    def exchange(m_stack):
        """Permute per-edge values: src-major layout -> dst-major layout."""
        m_flat = np.ascontiguousarray(m_stack).reshape(-1)
        v = np.zeros(NC * P * CD, exch_np)
        v[dslot] = m_flat[sslot]
        return v.reshape(NC, P, CD)

    ncA = build_pA(KDs, KSs)
    rA = run(ncA, [dict(maskD=maskD[kk], xg=x_grids[kk]) for kk in range(NC)],
             "pA")
    dinv_g = np.stack([rA[kk]["dinv"] for kk in range(NC)])
    u_g = np.stack([rA[kk]["u"] for kk in range(NC)])
    vD1 = exchange(np.stack([rA[kk]["m1"] for kk in range(NC)]))

    ncB = build_pB(KDs, KSs)
    rB = run(ncB, [dict(vD1=vD1[kk], u=u_g[kk], dinvg=dinv_g[kk])
                   for kk in range(NC)], "pB")
    y_g = np.stack([rB[kk]["yg"] for kk in range(NC)])
    vD2 = exchange(np.stack([rB[kk]["m2"] for kk in range(NC)]))

    ncC = build_pC(KDs, b2_zero=bool(np.all(b2 == 0.0)))
    rC = run(ncC, [dict(vD2=vD2[kk], dinvg=dinv_g[kk], yg=y_g[kk],
                        maskg=mask_grids[kk], cvec=cvec)
                   for kk in range(NC)], "pC")
    acc = np.stack([rC[kk]["acc"] for kk in range(NC)])

    pooled = acc.sum(axis=(0, 1)).astype(np.float64) / float(x.shape[0])
    logits = pooled @ Wl + bl
    m = logits.max()
    out = (logits - m) - np.log(np.exp(logits - m).sum())
    return out[None, :].astype(np.float32), phase_ns


def kernel(**inputs) -> np.ndarray:
    out, _ = run_pipeline(inputs, trace=False)
    return out


# revision 13
# speedup vs baseline: 1.8537x; 1.0230x over previous
"""Trainium2 Bass kernel for nn_Classifier_8461085573484 (2-layer GCN classifier).

Math: with x [N,1] and b1=0 (structurally true for this problem), both GCN
layers collapse to scalar per-node quantities:
  deg_d = indeg(d)+1;  dinv = 1/sqrt(deg);  u = x*dinv
  S_d   = sum_{e->d} u[src];   y = dinv^2*(S + u)   (y = layer1-scalar * dinv)
  sv_d  = sum_{e->d} y[src];  sp_d = sum_{e->d} relu(y[src]);  sm = sp - sv
  alpha = dinv*(sp + relu(y));      beta = dinv*(sm + relu(-y))
  out2  = relu(alpha a^T + beta b^T + b2), a = relu(W1)@W2, b = relu(-W1)@W2
  logits = mean(out2) @ Wl + bl -> log_softmax.

Sharding (8 NeuronCores): nodes are sorted by max(indeg, outdeg) and dealt
round-robin in groups of 1792 to (core, q-chunk); node slot (k, lane, q).
This makes each q-chunk degree-homogeneous, so the padded per-node edge
segments can use a per-chunk width = that chunk's max degree (~1.1x total
slots vs uniform-max padding ~1.7x), and gives every core identical DMA/
compute volume.

Layout: per-edge values live in *expanded row layout* grids [P, cols]:
node (lane, q) owns the KD_c-slot column segment at colD[q]; edge j of that
node (rank by dst or src) occupies slot j, pad slots are zero.  Segment sums
are then plain vector tensor_reduce over [P, QC, KD_c] (no one-hot work
blowup); the neighbor "gather" is a device-side broadcast of the node grid
into the src-major expanded layout (GpSimd engine, overlapping the Vector
reduces).  The host only routes / permutes per-edge values between the
src-major and dst-major layouts (no arithmetic) and applies the O(1)
classifier head.
"""
import contextlib
import ctypes
import sys
import types

import numpy as np

from concourse import bacc, bass, mybir
import concourse.tile as tile
from concourse import bass_utils

P = 128
Q = 98
NSH = P * Q            # 12544 nodes per NC shard
NC = 8
NPAD = NSH * NC        # 100352
N = 100000
F32 = mybir.dt.float32
BF16 = mybir.dt.bfloat16
QC = 14                # q-chunk size (Q = 7 chunks of 14)
NCH = Q // QC          # 7 chunks
G = P * QC             # 1792 nodes per (core, chunk)
EXCH_BF16 = True       # exchange per-edge values in bf16 (halves DMA traffic)


def _install_ntff_shim():
    """Provide antenv.axon_hooks so run_bass_kernel_spmd(trace=True) works."""
    if "antenv.axon_hooks" in sys.modules:
        return
    import antenv

    _hook = None
    try:
        lib = ctypes.CDLL("/opt/axon/libaxon_pjrt.so")
        if hasattr(lib, "axon_start_nrt_profile"):
            lib.axon_start_nrt_profile.argtypes = [
                ctypes.POINTER(ctypes.c_int64), ctypes.c_size_t]
            lib.axon_start_nrt_profile.restype = ctypes.c_int64
            lib.axon_stop_nrt_profile.argtypes = [ctypes.c_char_p]
            lib.axon_stop_nrt_profile.restype = ctypes.c_int64

            @contextlib.contextmanager
            def _hook_impl(output_dir, device_ids):
                import jax
                jax.devices()
                if device_ids:
                    ids = (ctypes.c_int64 * len(device_ids))(*device_ids)
                    rc = lib.axon_start_nrt_profile(ids, len(device_ids))
                else:
                    rc = lib.axon_start_nrt_profile(None, 0)
                if rc != 0:
                    raise RuntimeError(f"axon_start_nrt_profile rc={rc}")
                try:
                    yield
                finally:
                    n = lib.axon_stop_nrt_profile(str(output_dir).encode())
                    if n < 0:
                        raise RuntimeError(f"axon_stop_nrt_profile rc={n}")

            _hook = _hook_impl
    except OSError:
        pass

    mod = types.ModuleType("antenv.axon_hooks")
    mod._hook = _hook
    mod.get_axon_ntff_profile_hook = lambda: mod._hook

    def set_axon_ntff_profile_hook(h):
        mod._hook = h

    mod.set_axon_ntff_profile_hook = set_axon_ntff_profile_hook
    sys.modules["antenv.axon_hooks"] = mod
    antenv.axon_hooks = mod


_install_ntff_shim()


# ---------------- host routing (sharding/layout only, no arithmetic) -------

def _ranks(keys):
    """Rank of each edge within its node group."""
    counts = np.bincount(keys, minlength=NPAD).astype(np.int64)
    starts = np.zeros(NPAD, np.int64)
    starts[1:] = np.cumsum(counts)[:-1]
    order = np.argsort(keys, kind="stable")
    rank = np.empty(keys.shape[0], np.int64)
    rank[order] = np.arange(keys.shape[0], dtype=np.int64) - starts[keys[order]]
    return rank


def _node_layout(indeg, outdeg):
    """Degree-sorted node placement + per-chunk segment widths."""
    key = np.maximum(indeg, outdeg)
    order = np.argsort(-key, kind="stable")        # sorted pos -> node id
    i = np.arange(NPAD)
    g = i // G
    j = i - g * G
    core = g % NC
    chunk = g // NC
    lane = j % P
    q = chunk * QC + (j // P)
    core_of = np.empty(NPAD, np.int64)
    lane_of = np.empty(NPAD, np.int64)
    q_of = np.empty(NPAD, np.int64)
    core_of[order] = core
    lane_of[order] = lane
    q_of[order] = q
    sd = indeg[order].reshape(NCH, NC * G)
    so = outdeg[order].reshape(NCH, NC * G)
    KDs = [int(w) + (int(w) & 1) for w in sd.max(axis=1)]
    KSs = [int(w) + (int(w) & 1) for w in so.max(axis=1)]
    KDs = [max(w, 2) for w in KDs]
    KSs = [max(w, 2) for w in KSs]
    return core_of, lane_of, q_of, KDs, KSs


def _col_bases(Ks):
    """Column base per q for per-chunk widths Ks; returns (bases[Q], total)."""
    bases = np.zeros(Q, np.int64)
    off = 0
    for c, w in enumerate(Ks):
        for qq in range(QC):
            bases[c * QC + qq] = off + qq * w
        off += QC * w
    return bases, off


def _grid_scatter(core_of, lane_of, q_of, vec_padded):
    out = np.zeros((NC, P, Q), np.float32)
    out[core_of, lane_of, q_of] = vec_padded
    return out


# ---------------- device phase builders ----------------

def _exch_dt():
    return BF16 if EXCH_BF16 else F32


def _exch_np():
    import ml_dtypes
    return ml_dtypes.bfloat16 if EXCH_BF16 else np.float32


def build_pA(KDs, KSs):
    """indeg via mask row-reduce -> dinv, u; broadcast u to src-major m1."""
    EXT = _exch_dt()
    CD = QC * sum(KDs)
    CS = QC * sum(KSs)
    KDm, KSm = max(KDs), max(KSs)
    nc = bacc.Bacc("TRN2", target_bir_lowering=False, debug=False)
    maskD = nc.dram_tensor("maskD", [P, CD], BF16, kind="ExternalInput")
    xg = nc.dram_tensor("xg", [P, Q], F32, kind="ExternalInput")
    dinv_o = nc.dram_tensor("dinv", [P, Q], F32, kind="ExternalOutput")
    u_o = nc.dram_tensor("u", [P, Q], F32, kind="ExternalOutput")
    m1_o = nc.dram_tensor("m1", [P, CS], EXT, kind="ExternalOutput")
    with tile.TileContext(nc) as tc:
        with tc.tile_pool(name="sb", bufs=1) as pool, \
             tc.tile_pool(name="inp", bufs=7) as inp, \
             tc.tile_pool(name="outp", bufs=4) as outp:
            xg_sb = pool.tile([P, Q], F32, tag="xg")
            nc.sync.dma_start(xg_sb[:], xg.ap())
            indeg = pool.tile([P, Q], BF16, tag="indeg")
            dinv_sb = pool.tile([P, Q], F32, tag="dinv")
            u_sb = pool.tile([P, Q], F32, tag="u")
            sq_sb = pool.tile([P, Q], F32, tag="sq")
            ones_sb = pool.tile([P, QC * KSm], EXT, tag="ones")
            nc.gpsimd.memset(ones_sb[:], 1.0)
            offd = offs = 0
            for ci in range(NCH):
                kd, ks = KDs[ci], KSs[ci]
                c0 = ci * QC
                mt = inp.tile([P, QC * KDm], BF16, tag="mchunk")
                nc.sync.dma_start(mt[:, :QC * kd],
                                  maskD.ap()[:, offd:offd + QC * kd])
                with nc.allow_low_precision("integer counts exact in bf16"):
                    nc.vector.tensor_reduce(
                        out=indeg[:, c0:c0 + QC],
                        in_=mt[:, :QC * kd].rearrange("p (q k) -> p q k", k=kd),
                        axis=mybir.AxisListType.X, op=mybir.AluOpType.add)
                nc.scalar.activation(
                    out=sq_sb[:, c0:c0 + QC], in_=indeg[:, c0:c0 + QC],
                    func=mybir.ActivationFunctionType.Sqrt, bias=1.0, scale=1.0)
                nc.vector.reciprocal(out=dinv_sb[:, c0:c0 + QC],
                                     in_=sq_sb[:, c0:c0 + QC])
                nc.vector.tensor_tensor(
                    out=u_sb[:, c0:c0 + QC], in0=xg_sb[:, c0:c0 + QC],
                    in1=dinv_sb[:, c0:c0 + QC], op=mybir.AluOpType.mult)
                bt = outp.tile([P, QC * KSm], EXT, tag="bchunk")
                beng = nc.vector if ci in (2, 5) else nc.gpsimd
                beng.tensor_tensor(
                    out=bt[:, :QC * ks].rearrange("p (q k) -> p q k", k=ks),
                    in0=ones_sb[:, :QC * ks].rearrange("p (q k) -> p q k", k=ks),
                    in1=u_sb[:, c0:c0 + QC].rearrange(
                        "p (q one) -> p q one", one=1).to_broadcast([P, QC, ks]),
                    op=mybir.AluOpType.mult)
                nc.scalar.dma_start(m1_o.ap()[:, offs:offs + QC * ks],
                                    bt[:, :QC * ks])
                offd += QC * kd
                offs += QC * ks
            nc.scalar.dma_start(dinv_o.ap(), dinv_sb[:])
            nc.scalar.dma_start(u_o.ap(), u_sb[:])
    nc.compile()
    return nc


def build_pB(KDs, KSs):
    """S = segsum(vD1); y = dinv^2 * (S + u); broadcast y to src-major m2."""
    EXT = _exch_dt()
    CD = QC * sum(KDs)
    CS = QC * sum(KSs)
    KDm, KSm = max(KDs), max(KSs)
    nc = bacc.Bacc("TRN2", target_bir_lowering=False, debug=False)
    vD1 = nc.dram_tensor("vD1", [P, CD], EXT, kind="ExternalInput")
    u_i = nc.dram_tensor("u", [P, Q], F32, kind="ExternalInput")
    dinv_i = nc.dram_tensor("dinvg", [P, Q], F32, kind="ExternalInput")
    y_o = nc.dram_tensor("yg", [P, Q], F32, kind="ExternalOutput")
    m2_o = nc.dram_tensor("m2", [P, CS], EXT, kind="ExternalOutput")
    with tile.TileContext(nc) as tc:
        with tc.tile_pool(name="sb", bufs=1) as pool, \
             tc.tile_pool(name="inp", bufs=7) as inp, \
             tc.tile_pool(name="outp", bufs=4) as outp:
            u_sb = pool.tile([P, Q], F32, tag="u")
            dinv_sb = pool.tile([P, Q], F32, tag="dinv")
            d2_sb = pool.tile([P, Q], F32, tag="d2")
            s_sb = pool.tile([P, Q], BF16, tag="s")
            y_sb = pool.tile([P, Q], F32, tag="y")
            nc.sync.dma_start(u_sb[:], u_i.ap())
            nc.sync.dma_start(dinv_sb[:], dinv_i.ap())
            nc.vector.tensor_tensor(out=d2_sb[:], in0=dinv_sb[:],
                                    in1=dinv_sb[:], op=mybir.AluOpType.mult)
            ones_sb = pool.tile([P, QC * KSm], EXT, tag="ones")
            nc.gpsimd.memset(ones_sb[:], 1.0)
            offd = offs = 0
            for ci in range(NCH):
                kd, ks = KDs[ci], KSs[ci]
                c0 = ci * QC
                vt = inp.tile([P, QC * KDm], EXT, tag="vchunk")
                nc.sync.dma_start(vt[:, :QC * kd],
                                  vD1.ap()[:, offd:offd + QC * kd])
                with nc.allow_low_precision("segment sums tolerate bf16"):
                    nc.vector.tensor_reduce(
                        out=s_sb[:, c0:c0 + QC],
                        in_=vt[:, :QC * kd].rearrange("p (q k) -> p q k", k=kd),
                        axis=mybir.AxisListType.X, op=mybir.AluOpType.add)
                nc.vector.tensor_tensor(
                    out=y_sb[:, c0:c0 + QC], in0=s_sb[:, c0:c0 + QC],
                    in1=u_sb[:, c0:c0 + QC], op=mybir.AluOpType.add)
                nc.vector.tensor_tensor(
                    out=y_sb[:, c0:c0 + QC], in0=y_sb[:, c0:c0 + QC],
                    in1=d2_sb[:, c0:c0 + QC], op=mybir.AluOpType.mult)
                bt = outp.tile([P, QC * KSm], EXT, tag="bchunk")
                beng = nc.vector if ci in (2, 5) else nc.gpsimd
                beng.tensor_tensor(
                    out=bt[:, :QC * ks].rearrange("p (q k) -> p q k", k=ks),
                    in0=ones_sb[:, :QC * ks].rearrange("p (q k) -> p q k", k=ks),
                    in1=y_sb[:, c0:c0 + QC].rearrange(
                        "p (q one) -> p q one", one=1).to_broadcast([P, QC, ks]),
                    op=mybir.AluOpType.mult)
                nc.scalar.dma_start(m2_o.ap()[:, offs:offs + QC * ks],
                                    bt[:, :QC * ks])
                offd += QC * kd
                offs += QC * ks
            nc.scalar.dma_start(y_o.ap(), y_sb[:])
    nc.compile()
    return nc


def build_pC(KDs, b2_zero):
    """sp/sm segsums of relu'd y messages; alpha/beta; 16-feature sums."""
    EXT = _exch_dt()
    CD = QC * sum(KDs)
    KDm = max(KDs)
    nc = bacc.Bacc("TRN2", target_bir_lowering=False, debug=False)
    vD2 = nc.dram_tensor("vD2", [P, CD], EXT, kind="ExternalInput")
    dinv_i = nc.dram_tensor("dinvg", [P, Q], F32, kind="ExternalInput")
    y_i = nc.dram_tensor("yg", [P, Q], F32, kind="ExternalInput")
    mask_i = nc.dram_tensor("maskg", [P, Q], F32, kind="ExternalInput")
    cvec = nc.dram_tensor("cvec", [P, 48], F32, kind="ExternalInput")
    acc_o = nc.dram_tensor("acc", [P, 16], F32, kind="ExternalOutput")
    with tile.TileContext(nc) as tc:
        with tc.tile_pool(name="sb", bufs=1) as pool, \
             tc.tile_pool(name="inp", bufs=7) as inp, \
             tc.tile_pool(name="rel", bufs=4) as relp:
            dinv_sb = pool.tile([P, Q], F32, tag="dinv")
            y_sb = pool.tile([P, Q], F32, tag="y")
            mask_sb = pool.tile([P, Q], F32, tag="mask")
            cvec_sb = pool.tile([P, 48], F32, tag="cvec")
            sv_sb = pool.tile([P, Q], BF16, tag="sv")
            sp_sb = pool.tile([P, Q], BF16, tag="sp")
            for t_sb, t in ((dinv_sb, dinv_i), (y_sb, y_i),
                            (mask_sb, mask_i), (cvec_sb, cvec)):
                nc.sync.dma_start(t_sb[:], t.ap())
            offd = 0
            for ci in range(NCH):
                kd = KDs[ci]
                c0 = ci * QC
                vt = inp.tile([P, QC * KDm], EXT, tag="vchunk")
                nc.sync.dma_start(vt[:, :QC * kd],
                                  vD2.ap()[:, offd:offd + QC * kd])
                with nc.allow_low_precision("segment sums tolerate bf16"):
                    nc.vector.tensor_reduce(
                        out=sv_sb[:, c0:c0 + QC],
                        in_=vt[:, :QC * kd].rearrange("p (q k) -> p q k", k=kd),
                        axis=mybir.AxisListType.X, op=mybir.AluOpType.add)
                rt = relp.tile([P, QC * KDm], EXT, tag="rchunk")
                nc.scalar.activation(out=rt[:, :QC * kd], in_=vt[:, :QC * kd],
                                     func=mybir.ActivationFunctionType.Relu)
                with nc.allow_low_precision("segment sums tolerate bf16"):
                    nc.vector.tensor_reduce(
                        out=sp_sb[:, c0:c0 + QC],
                        in_=rt[:, :QC * kd].rearrange("p (q k) -> p q k", k=kd),
                        axis=mybir.AxisListType.X, op=mybir.AluOpType.add)
                offd += QC * kd
            # node-side terms; mask folded into the dinv multiplier (valid
            # because pad nodes then contribute relu(b2)=0 when b2 == 0;
            # the b2 != 0 case keeps the explicit mask path below)
            yp = pool.tile([P, Q], F32, tag="yp")
            ym = pool.tile([P, Q], F32, tag="ym")
            alpha = pool.tile([P, Q], F32, tag="alpha")
            beta = pool.tile([P, Q], F32, tag="beta")
            dm = pool.tile([P, Q], F32, tag="dm")
            if b2_zero:
                nc.vector.tensor_tensor(out=dm[:], in0=dinv_sb[:],
                                        in1=mask_sb[:], op=mybir.AluOpType.mult)
            else:
                nc.scalar.activation(out=dm[:], in_=dinv_sb[:],
                                     func=mybir.ActivationFunctionType.Copy)
            nc.scalar.activation(out=yp[:], in_=y_sb[:],
                                 func=mybir.ActivationFunctionType.Relu)
            nc.vector.tensor_tensor(out=ym[:], in0=yp[:], in1=y_sb[:],
                                    op=mybir.AluOpType.subtract)
            nc.vector.tensor_tensor(out=alpha[:], in0=sp_sb[:], in1=yp[:],
                                    op=mybir.AluOpType.add)
            nc.vector.tensor_tensor(out=alpha[:], in0=alpha[:], in1=dm[:],
                                    op=mybir.AluOpType.mult)
            nc.vector.tensor_tensor(out=sv_sb[:], in0=sp_sb[:], in1=sv_sb[:],
                                    op=mybir.AluOpType.subtract)  # sm
            nc.vector.tensor_tensor(out=beta[:], in0=sv_sb[:], in1=ym[:],
                                    op=mybir.AluOpType.add)
            nc.vector.tensor_tensor(out=beta[:], in0=beta[:], in1=dm[:],
                                    op=mybir.AluOpType.mult)
            # 16 features at once in [P, 16, Q] layout
            t1 = pool.tile([P, 16 * Q], F32, tag="t1")
            t2 = pool.tile([P, 16 * Q], F32, tag="t2")
            t13 = t1[:].rearrange("p (j q) -> p j q", j=16)
            t23 = t2[:].rearrange("p (j q) -> p j q", j=16)
            alpha_b = alpha[:].rearrange("p (one q) -> p one q",
                                         one=1).to_broadcast([P, 16, Q])
            beta_b = beta[:].rearrange("p (one q) -> p one q",
                                       one=1).to_broadcast([P, 16, Q])
            mask_b = mask_sb[:].rearrange("p (one q) -> p one q",
                                          one=1).to_broadcast([P, 16, Q])
            a_b = cvec_sb[:, 0:16].rearrange("p (j one) -> p j one",
                                             one=1).to_broadcast([P, 16, Q])
            b_b = cvec_sb[:, 16:32].rearrange("p (j one) -> p j one",
                                              one=1).to_broadcast([P, 16, Q])
            b2_b = cvec_sb[:, 32:48].rearrange("p (j one) -> p j one",
                                               one=1).to_broadcast([P, 16, Q])
            nc.vector.tensor_tensor(out=t13, in0=alpha_b, in1=a_b,
                                    op=mybir.AluOpType.mult)
            nc.gpsimd.tensor_tensor(out=t23, in0=beta_b, in1=b_b,
                                    op=mybir.AluOpType.mult)
            nc.vector.tensor_tensor(out=t13, in0=t13, in1=t23,
                                    op=mybir.AluOpType.add)
            if not b2_zero:
                nc.vector.tensor_tensor(out=t13, in0=t13, in1=b2_b,
                                        op=mybir.AluOpType.add)
            nc.scalar.activation(out=t1[:], in_=t1[:],
                                 func=mybir.ActivationFunctionType.Relu)
            if not b2_zero:
                nc.vector.tensor_tensor(out=t13, in0=t13, in1=mask_b,
                                        op=mybir.AluOpType.mult)
            acc_sb = pool.tile([P, 16], F32, tag="acc")
            nc.vector.tensor_reduce(out=acc_sb[:], in_=t13,
                                    axis=mybir.AxisListType.X,
                                    op=mybir.AluOpType.add)
            nc.scalar.dma_start(acc_o.ap(), acc_sb[:])
    nc.compile()
    return nc


# ---------------- pipeline ----------------

def run_pipeline(inputs, trace=False):
    import ml_dtypes
    exch_np = ml_dtypes.bfloat16 if EXCH_BF16 else np.float32

    x = np.asarray(inputs["x"]).reshape(-1).astype(np.float32)
    ei = np.asarray(inputs["edge_index"])
    src = ei[0].astype(np.int64)
    dst = ei[1].astype(np.int64)
    W1 = np.asarray(inputs["W1"]).astype(np.float64)[0]
    W2 = np.asarray(inputs["W2"]).astype(np.float64)
    b2 = np.asarray(inputs["b2"]).astype(np.float64)
    Wl = np.asarray(inputs["Wl"]).astype(np.float64)
    bl = np.asarray(inputs["bl"]).astype(np.float64)
    a_vec = np.maximum(W1, 0) @ W2
    b_vec = np.maximum(-W1, 0) @ W2

    xpad = np.zeros(NPAD, np.float32)
    xpad[:x.shape[0]] = x
    maskpad = np.zeros(NPAD, np.float32)
    maskpad[:x.shape[0]] = 1.0

    indeg_cnt = np.bincount(dst, minlength=NPAD).astype(np.int64)
    outdeg_cnt = np.bincount(src, minlength=NPAD).astype(np.int64)
    core_of, lane_of, q_of, KDs, KSs = _node_layout(indeg_cnt, outdeg_cnt)
    colD, CD = _col_bases(KDs)
    colS, CS = _col_bases(KSs)

    rank_d = _ranks(dst)
    rank_s = _ranks(src)
    dslot = (core_of[dst] * P + lane_of[dst]) * CD + colD[q_of[dst]] + rank_d
    sslot = (core_of[src] * P + lane_of[src]) * CS + colS[q_of[src]] + rank_s

    x_grids = _grid_scatter(core_of, lane_of, q_of, xpad)
    mask_grids = _grid_scatter(core_of, lane_of, q_of, maskpad)

    maskD = np.zeros(NC * P * CD, np.float32)
    maskD[dslot] = 1.0
    maskD = np.ascontiguousarray(
        maskD.reshape(NC, P, CD).astype(ml_dtypes.bfloat16))

    cvec = np.zeros(48, np.float32)
    cvec[0:16] = a_vec
    cvec[16:32] = b_vec
    cvec[32:48] = b2
    cvec = np.ascontiguousarray(np.tile(cvec, (P, 1)))

    phase_ns = {}

    def run(nc, in_maps, name):
        res = bass_utils.run_bass_kernel_spmd(
            nc, in_maps, core_ids=list(range(NC)), trace=trace)
        phase_ns[name] = res.exec_time_ns
        return res.results

    def exchange(m_stack):
        """Permute per-edge values: src-major layout -> dst-major layout."""
        m_flat = np.ascontiguousarray(m_stack).reshape(-1)
        v = np.zeros(NC * P * CD, exch_np)
        v[dslot] = m_flat[sslot]
        return v.reshape(NC, P, CD)

    ncA = build_pA(KDs, KSs)
    rA = run(ncA, [dict(maskD=maskD[kk], xg=x_grids[kk]) for kk in range(NC)],
             "pA")
    dinv_g = np.stack([rA[kk]["dinv"] for kk in range(NC)])
    u_g = np.stack([rA[kk]["u"] for kk in range(NC)])
    vD1 = exchange(np.stack([rA[kk]["m1"] for kk in range(NC)]))

    ncB = build_pB(KDs, KSs)
    rB = run(ncB, [dict(vD1=vD1[kk], u=u_g[kk], dinvg=dinv_g[kk])
                   for kk in range(NC)], "pB")
    y_g = np.stack([rB[kk]["yg"] for kk in range(NC)])
    vD2 = exchange(np.stack([rB[kk]["m2"] for kk in range(NC)]))

    ncC = build_pC(KDs, b2_zero=bool(np.all(b2 == 0.0)))
    rC = run(ncC, [dict(vD2=vD2[kk], dinvg=dinv_g[kk], yg=y_g[kk],
                        maskg=mask_grids[kk], cvec=cvec)
                   for kk in range(NC)], "pC")
    acc = np.stack([rC[kk]["acc"] for kk in range(NC)])

    pooled = acc.sum(axis=(0, 1)).astype(np.float64) / float(x.shape[0])
    logits = pooled @ Wl + bl
    m = logits.max()
    out = (logits - m) - np.log(np.exp(logits - m).sum())
    return out[None, :].astype(np.float32), phase_ns


def kernel(**inputs) -> np.ndarray:
    out, _ = run_pipeline(inputs, trace=False)
    return out


# revision 14
# speedup vs baseline: 1.8638x; 1.0055x over previous
"""Trainium2 Bass kernel for nn_Classifier_8461085573484 (2-layer GCN classifier).

Math: with x [N,1] and b1=0 (structurally true for this problem), both GCN
layers collapse to scalar per-node quantities:
  deg_d = indeg(d)+1;  dinv = 1/sqrt(deg);  u = x*dinv
  S_d   = sum_{e->d} u[src];   y = dinv^2*(S + u)   (y = layer1-scalar * dinv)
  sv_d  = sum_{e->d} y[src];  sp_d = sum_{e->d} relu(y[src]);  sm = sp - sv
  alpha = dinv*(sp + relu(y));      beta = dinv*(sm + relu(-y))
  out2  = relu(alpha a^T + beta b^T + b2), a = relu(W1)@W2, b = relu(-W1)@W2
  logits = mean(out2) @ Wl + bl -> log_softmax.

Sharding (8 NeuronCores): nodes are sorted by max(indeg, outdeg) and dealt
round-robin in groups of 1792 to (core, q-chunk); node slot (k, lane, q).
This makes each q-chunk degree-homogeneous, so the padded per-node edge
segments can use a per-chunk width = that chunk's max degree (~1.1x total
slots vs uniform-max padding ~1.7x), and gives every core identical DMA/
compute volume.

Layout: per-edge values live in *expanded row layout* grids [P, cols]:
node (lane, q) owns the KD_c-slot column segment at colD[q]; edge j of that
node (rank by dst or src) occupies slot j, pad slots are zero.  Segment sums
are then plain vector tensor_reduce over [P, QC, KD_c] (no one-hot work
blowup); the neighbor "gather" is a device-side broadcast of the node grid
into the src-major expanded layout (GpSimd engine, overlapping the Vector
reduces).  The host only routes / permutes per-edge values between the
src-major and dst-major layouts (no arithmetic) and applies the O(1)
classifier head.
"""
import contextlib
import ctypes
import sys
import types

import numpy as np

from concourse import bacc, bass, mybir
import concourse.tile as tile
from concourse import bass_utils

P = 128
Q = 98
NSH = P * Q            # 12544 nodes per NC shard
NC = 8
NPAD = NSH * NC        # 100352
N = 100000
F32 = mybir.dt.float32
BF16 = mybir.dt.bfloat16
QC = 14                # q-chunk size (Q = 7 chunks of 14)
NCH = Q // QC          # 7 chunks
G = P * QC             # 1792 nodes per (core, chunk)
EXCH_BF16 = True       # exchange per-edge values in bf16 (halves DMA traffic)


def _install_ntff_shim():
    """Provide antenv.axon_hooks so run_bass_kernel_spmd(trace=True) works."""
    if "antenv.axon_hooks" in sys.modules:
        return
    import antenv

    _hook = None
    try:
        lib = ctypes.CDLL("/opt/axon/libaxon_pjrt.so")
        if hasattr(lib, "axon_start_nrt_profile"):
            lib.axon_start_nrt_profile.argtypes = [
                ctypes.POINTER(ctypes.c_int64), ctypes.c_size_t]
            lib.axon_start_nrt_profile.restype = ctypes.c_int64
            lib.axon_stop_nrt_profile.argtypes = [ctypes.c_char_p]
            lib.axon_stop_nrt_profile.restype = ctypes.c_int64

            @contextlib.contextmanager
            def _hook_impl(output_dir, device_ids):
                import jax
                jax.devices()
                if device_ids:
                    ids = (ctypes.c_int64 * len(device_ids))(*device_ids)
                    rc = lib.axon_start_nrt_profile(ids, len(device_ids))
                else:
                    rc = lib.axon_start_nrt_profile(None, 0)
                if rc != 0:
                    raise RuntimeError(f"axon_start_nrt_profile rc={rc}")
                try:
                    yield
                finally:
                    n = lib.axon_stop_nrt_profile(str(output_dir).encode())
                    if n < 0:
                        raise RuntimeError(f"axon_stop_nrt_profile rc={n}")

            _hook = _hook_impl
    except OSError:
        pass

    mod = types.ModuleType("antenv.axon_hooks")
    mod._hook = _hook
    mod.get_axon_ntff_profile_hook = lambda: mod._hook

    def set_axon_ntff_profile_hook(h):
        mod._hook = h

    mod.set_axon_ntff_profile_hook = set_axon_ntff_profile_hook
    sys.modules["antenv.axon_hooks"] = mod
    antenv.axon_hooks = mod


_install_ntff_shim()


# ---------------- host routing (sharding/layout only, no arithmetic) -------

def _ranks(keys):
    """Rank of each edge within its node group."""
    counts = np.bincount(keys, minlength=NPAD).astype(np.int64)
    starts = np.zeros(NPAD, np.int64)
    starts[1:] = np.cumsum(counts)[:-1]
    order = np.argsort(keys, kind="stable")
    rank = np.empty(keys.shape[0], np.int64)
    rank[order] = np.arange(keys.shape[0], dtype=np.int64) - starts[keys[order]]
    return rank


def _node_layout(indeg, outdeg):
    """Degree-sorted node placement + per-chunk segment widths."""
    key = np.maximum(indeg, outdeg)
    order = np.argsort(-key, kind="stable")        # sorted pos -> node id
    i = np.arange(NPAD)
    g = i // G
    j = i - g * G
    core = g % NC
    chunk = g // NC
    lane = j % P
    q = chunk * QC + (j // P)
    core_of = np.empty(NPAD, np.int64)
    lane_of = np.empty(NPAD, np.int64)
    q_of = np.empty(NPAD, np.int64)
    core_of[order] = core
    lane_of[order] = lane
    q_of[order] = q
    sd = indeg[order].reshape(NCH, NC * G)
    so = outdeg[order].reshape(NCH, NC * G)
    KDs = [int(w) + (int(w) & 1) for w in sd.max(axis=1)]
    KSs = [int(w) + (int(w) & 1) for w in so.max(axis=1)]
    KDs = [max(w, 2) for w in KDs]
    KSs = [max(w, 2) for w in KSs]
    return core_of, lane_of, q_of, KDs, KSs


def _col_bases(Ks):
    """Column base per q for per-chunk widths Ks; returns (bases[Q], total)."""
    bases = np.zeros(Q, np.int64)
    off = 0
    for c, w in enumerate(Ks):
        for qq in range(QC):
            bases[c * QC + qq] = off + qq * w
        off += QC * w
    return bases, off


def _grid_scatter(core_of, lane_of, q_of, vec_padded):
    out = np.zeros((NC, P, Q), np.float32)
    out[core_of, lane_of, q_of] = vec_padded
    return out


# ---------------- device phase builders ----------------

def _exch_dt():
    return BF16 if EXCH_BF16 else F32


def _exch_np():
    import ml_dtypes
    return ml_dtypes.bfloat16 if EXCH_BF16 else np.float32


def build_pA(KDs, KSs):
    """indeg via mask row-reduce -> dinv, u; broadcast u to src-major m1."""
    EXT = _exch_dt()
    CD = QC * sum(KDs)
    CS = QC * sum(KSs)
    KDm, KSm = max(KDs), max(KSs)
    nc = bacc.Bacc("TRN2", target_bir_lowering=False, debug=False)
    maskD = nc.dram_tensor("maskD", [P, CD], BF16, kind="ExternalInput")
    xg = nc.dram_tensor("xg", [P, Q], F32, kind="ExternalInput")
    dinv_o = nc.dram_tensor("dinv", [P, Q], F32, kind="ExternalOutput")
    u_o = nc.dram_tensor("u", [P, Q], F32, kind="ExternalOutput")
    m1_o = nc.dram_tensor("m1", [P, CS], EXT, kind="ExternalOutput")
    with tile.TileContext(nc) as tc:
        with tc.tile_pool(name="sb", bufs=1) as pool, \
             tc.tile_pool(name="inp", bufs=7) as inp, \
             tc.tile_pool(name="outp", bufs=4) as outp:
            xg_sb = pool.tile([P, Q], F32, tag="xg")
            nc.sync.dma_start(xg_sb[:], xg.ap())
            indeg = pool.tile([P, Q], BF16, tag="indeg")
            dinv_sb = pool.tile([P, Q], F32, tag="dinv")
            u_sb = pool.tile([P, Q], F32, tag="u")
            sq_sb = pool.tile([P, Q], F32, tag="sq")
            ones_sb = pool.tile([P, QC * KSm], EXT, tag="ones")
            nc.gpsimd.memset(ones_sb[:], 1.0)
            offd = offs = 0
            for ci in range(NCH):
                kd, ks = KDs[ci], KSs[ci]
                c0 = ci * QC
                mt = inp.tile([P, QC * KDm], BF16, tag="mchunk")
                nc.sync.dma_start(mt[:, :QC * kd],
                                  maskD.ap()[:, offd:offd + QC * kd])
                with nc.allow_low_precision("integer counts exact in bf16"):
                    nc.vector.tensor_reduce(
                        out=indeg[:, c0:c0 + QC],
                        in_=mt[:, :QC * kd].rearrange("p (q k) -> p q k", k=kd),
                        axis=mybir.AxisListType.X, op=mybir.AluOpType.add)
                nc.scalar.activation(
                    out=sq_sb[:, c0:c0 + QC], in_=indeg[:, c0:c0 + QC],
                    func=mybir.ActivationFunctionType.Sqrt, bias=1.0, scale=1.0)
                nc.vector.reciprocal(out=dinv_sb[:, c0:c0 + QC],
                                     in_=sq_sb[:, c0:c0 + QC])
                nc.vector.tensor_tensor(
                    out=u_sb[:, c0:c0 + QC], in0=xg_sb[:, c0:c0 + QC],
                    in1=dinv_sb[:, c0:c0 + QC], op=mybir.AluOpType.mult)
                bt = outp.tile([P, QC * KSm], EXT, tag="bchunk")
                beng = nc.vector if ci in (2, 5) else nc.gpsimd
                beng.tensor_tensor(
                    out=bt[:, :QC * ks].rearrange("p (q k) -> p q k", k=ks),
                    in0=ones_sb[:, :QC * ks].rearrange("p (q k) -> p q k", k=ks),
                    in1=u_sb[:, c0:c0 + QC].rearrange(
                        "p (q one) -> p q one", one=1).to_broadcast([P, QC, ks]),
                    op=mybir.AluOpType.mult)
                nc.scalar.dma_start(m1_o.ap()[:, offs:offs + QC * ks],
                                    bt[:, :QC * ks])
                offd += QC * kd
                offs += QC * ks
            nc.scalar.dma_start(dinv_o.ap(), dinv_sb[:])
            nc.scalar.dma_start(u_o.ap(), u_sb[:])
    nc.compile()
    return nc


def build_pB(KDs, KSs):
    """S = segsum(vD1); y = dinv^2 * (S + u); broadcast y to src-major m2."""
    EXT = _exch_dt()
    CD = QC * sum(KDs)
    CS = QC * sum(KSs)
    KDm, KSm = max(KDs), max(KSs)
    nc = bacc.Bacc("TRN2", target_bir_lowering=False, debug=False)
    vD1 = nc.dram_tensor("vD1", [P, CD], EXT, kind="ExternalInput")
    u_i = nc.dram_tensor("u", [P, Q], F32, kind="ExternalInput")
    dinv_i = nc.dram_tensor("dinvg", [P, Q], F32, kind="ExternalInput")
    y_o = nc.dram_tensor("yg", [P, Q], F32, kind="ExternalOutput")
    m2_o = nc.dram_tensor("m2", [P, CS], EXT, kind="ExternalOutput")
    with tile.TileContext(nc) as tc:
        with tc.tile_pool(name="sb", bufs=1) as pool, \
             tc.tile_pool(name="inp", bufs=7) as inp, \
             tc.tile_pool(name="outp", bufs=4) as outp:
            u_sb = pool.tile([P, Q], F32, tag="u")
            dinv_sb = pool.tile([P, Q], F32, tag="dinv")
            d2_sb = pool.tile([P, Q], F32, tag="d2")
            s_sb = pool.tile([P, Q], BF16, tag="s")
            y_sb = pool.tile([P, Q], F32, tag="y")
            nc.sync.dma_start(u_sb[:], u_i.ap())
            nc.sync.dma_start(dinv_sb[:], dinv_i.ap())
            nc.vector.tensor_tensor(out=d2_sb[:], in0=dinv_sb[:],
                                    in1=dinv_sb[:], op=mybir.AluOpType.mult)
            ones_sb = pool.tile([P, QC * KSm], EXT, tag="ones")
            nc.gpsimd.memset(ones_sb[:], 1.0)
            offd = offs = 0
            for ci in range(NCH):
                kd, ks = KDs[ci], KSs[ci]
                c0 = ci * QC
                vt = inp.tile([P, QC * KDm], EXT, tag="vchunk")
                nc.sync.dma_start(vt[:, :QC * kd],
                                  vD1.ap()[:, offd:offd + QC * kd])
                with nc.allow_low_precision("segment sums tolerate bf16"):
                    nc.vector.tensor_reduce(
                        out=s_sb[:, c0:c0 + QC],
                        in_=vt[:, :QC * kd].rearrange("p (q k) -> p q k", k=kd),
                        axis=mybir.AxisListType.X, op=mybir.AluOpType.add)
                nc.vector.tensor_tensor(
                    out=y_sb[:, c0:c0 + QC], in0=s_sb[:, c0:c0 + QC],
                    in1=u_sb[:, c0:c0 + QC], op=mybir.AluOpType.add)
                nc.vector.tensor_tensor(
                    out=y_sb[:, c0:c0 + QC], in0=y_sb[:, c0:c0 + QC],
                    in1=d2_sb[:, c0:c0 + QC], op=mybir.AluOpType.mult)
                bt = outp.tile([P, QC * KSm], EXT, tag="bchunk")
                beng = nc.vector if ci in (2, 5) else nc.gpsimd
                beng.tensor_tensor(
                    out=bt[:, :QC * ks].rearrange("p (q k) -> p q k", k=ks),
                    in0=ones_sb[:, :QC * ks].rearrange("p (q k) -> p q k", k=ks),
                    in1=y_sb[:, c0:c0 + QC].rearrange(
                        "p (q one) -> p q one", one=1).to_broadcast([P, QC, ks]),
                    op=mybir.AluOpType.mult)
                nc.scalar.dma_start(m2_o.ap()[:, offs:offs + QC * ks],
                                    bt[:, :QC * ks])
                offd += QC * kd
                offs += QC * ks
            nc.scalar.dma_start(y_o.ap(), y_sb[:])
    nc.compile()
    return nc


def build_pC(KDs, b2_zero):
    """sp/sm segsums of relu'd y messages; alpha/beta; 16-feature sums."""
    EXT = _exch_dt()
    CD = QC * sum(KDs)
    KDm = max(KDs)
    nc = bacc.Bacc("TRN2", target_bir_lowering=False, debug=False)
    vD2 = nc.dram_tensor("vD2", [P, CD], EXT, kind="ExternalInput")
    dinv_i = nc.dram_tensor("dinvg", [P, Q], F32, kind="ExternalInput")
    y_i = nc.dram_tensor("yg", [P, Q], F32, kind="ExternalInput")
    mask_i = nc.dram_tensor("maskg", [P, Q], F32, kind="ExternalInput")
    cvec = nc.dram_tensor("cvec", [P, 48], F32, kind="ExternalInput")
    acc_o = nc.dram_tensor("acc", [P, 16], F32, kind="ExternalOutput")
    with tile.TileContext(nc) as tc:
        with tc.tile_pool(name="sb", bufs=1) as pool, \
             tc.tile_pool(name="inp", bufs=7) as inp, \
             tc.tile_pool(name="rel", bufs=4) as relp:
            dinv_sb = pool.tile([P, Q], F32, tag="dinv")
            y_sb = pool.tile([P, Q], F32, tag="y")
            mask_sb = pool.tile([P, Q], F32, tag="mask")
            cvec_sb = pool.tile([P, 48], F32, tag="cvec")
            sv_sb = pool.tile([P, Q], BF16, tag="sv")
            sp_sb = pool.tile([P, Q], BF16, tag="sp")
            for t_sb, t in ((dinv_sb, dinv_i), (y_sb, y_i),
                            (mask_sb, mask_i), (cvec_sb, cvec)):
                nc.sync.dma_start(t_sb[:], t.ap())
            offd = 0
            for ci in range(NCH):
                kd = KDs[ci]
                c0 = ci * QC
                vt = inp.tile([P, QC * KDm], EXT, tag="vchunk")
                nc.sync.dma_start(vt[:, :QC * kd],
                                  vD2.ap()[:, offd:offd + QC * kd])
                with nc.allow_low_precision("segment sums tolerate bf16"):
                    nc.vector.tensor_reduce(
                        out=sv_sb[:, c0:c0 + QC],
                        in_=vt[:, :QC * kd].rearrange("p (q k) -> p q k", k=kd),
                        axis=mybir.AxisListType.X, op=mybir.AluOpType.add)
                rt = relp.tile([P, QC * KDm], EXT, tag="rchunk")
                nc.scalar.activation(out=rt[:, :QC * kd], in_=vt[:, :QC * kd],
                                     func=mybir.ActivationFunctionType.Relu)
                with nc.allow_low_precision("segment sums tolerate bf16"):
                    nc.vector.tensor_reduce(
                        out=sp_sb[:, c0:c0 + QC],
                        in_=rt[:, :QC * kd].rearrange("p (q k) -> p q k", k=kd),
                        axis=mybir.AxisListType.X, op=mybir.AluOpType.add)
                offd += QC * kd
            # node-side terms; mask folded into the dinv multiplier (valid
            # because pad nodes then contribute relu(b2)=0 when b2 == 0;
            # the b2 != 0 case keeps the explicit mask path below).
            # Processed in two column halves so half 1 overlaps the tail
            # chunks' DMA/reduces.
            yp = pool.tile([P, Q], F32, tag="yp")
            ym = pool.tile([P, Q], F32, tag="ym")
            alpha = pool.tile([P, Q], F32, tag="alpha")
            beta = pool.tile([P, Q], F32, tag="beta")
            dm = pool.tile([P, Q], F32, tag="dm")
            if b2_zero:
                nc.vector.tensor_tensor(out=dm[:], in0=dinv_sb[:],
                                        in1=mask_sb[:], op=mybir.AluOpType.mult)
            else:
                nc.scalar.activation(out=dm[:], in_=dinv_sb[:],
                                     func=mybir.ActivationFunctionType.Copy)
            t1 = pool.tile([P, 16 * Q], F32, tag="t1")
            t2 = pool.tile([P, 16 * Q], F32, tag="t2")
            acc_sb = pool.tile([P, 16], F32, tag="acc")
            acc2_sb = pool.tile([P, 16], F32, tag="acc2")
            a_col = cvec_sb[:, 0:16].rearrange("p (j one) -> p j one", one=1)
            b_col = cvec_sb[:, 16:32].rearrange("p (j one) -> p j one", one=1)
            b2_col = cvec_sb[:, 32:48].rearrange("p (j one) -> p j one", one=1)
            halves = [(0, 4 * QC, acc_sb), (4 * QC, Q - 4 * QC, acc2_sb)]
            for h0, hw_, acc_t in halves:
                sl = slice(h0, h0 + hw_)
                nc.scalar.activation(out=yp[:, sl], in_=y_sb[:, sl],
                                     func=mybir.ActivationFunctionType.Relu)
                nc.gpsimd.tensor_tensor(out=ym[:, sl], in0=yp[:, sl],
                                        in1=y_sb[:, sl],
                                        op=mybir.AluOpType.subtract)
                nc.vector.tensor_tensor(out=alpha[:, sl], in0=sp_sb[:, sl],
                                        in1=yp[:, sl], op=mybir.AluOpType.add)
                nc.vector.tensor_tensor(out=alpha[:, sl], in0=alpha[:, sl],
                                        in1=dm[:, sl], op=mybir.AluOpType.mult)
                nc.gpsimd.tensor_tensor(out=sv_sb[:, sl], in0=sp_sb[:, sl],
                                        in1=sv_sb[:, sl],
                                        op=mybir.AluOpType.subtract)  # sm
                nc.vector.tensor_tensor(out=beta[:, sl], in0=sv_sb[:, sl],
                                        in1=ym[:, sl], op=mybir.AluOpType.add)
                nc.vector.tensor_tensor(out=beta[:, sl], in0=beta[:, sl],
                                        in1=dm[:, sl], op=mybir.AluOpType.mult)
                t13 = t1[:, h0 * 16:(h0 + hw_) * 16].rearrange(
                    "p (j q) -> p j q", j=16)
                t23 = t2[:, h0 * 16:(h0 + hw_) * 16].rearrange(
                    "p (j q) -> p j q", j=16)
                alpha_b = alpha[:, sl].rearrange(
                    "p (one q) -> p one q", one=1).to_broadcast([P, 16, hw_])
                beta_b = beta[:, sl].rearrange(
                    "p (one q) -> p one q", one=1).to_broadcast([P, 16, hw_])
                nc.vector.tensor_tensor(out=t13, in0=alpha_b,
                                        in1=a_col.to_broadcast([P, 16, hw_]),
                                        op=mybir.AluOpType.mult)
                nc.gpsimd.tensor_tensor(out=t23, in0=beta_b,
                                        in1=b_col.to_broadcast([P, 16, hw_]),
                                        op=mybir.AluOpType.mult)
                nc.vector.tensor_tensor(out=t13, in0=t13, in1=t23,
                                        op=mybir.AluOpType.add)
                if not b2_zero:
                    nc.vector.tensor_tensor(
                        out=t13, in0=t13,
                        in1=b2_col.to_broadcast([P, 16, hw_]),
                        op=mybir.AluOpType.add)
                nc.scalar.activation(out=t1[:, h0 * 16:(h0 + hw_) * 16],
                                     in_=t1[:, h0 * 16:(h0 + hw_) * 16],
                                     func=mybir.ActivationFunctionType.Relu)
                if not b2_zero:
                    mask_b = mask_sb[:, sl].rearrange(
                        "p (one q) -> p one q", one=1).to_broadcast([P, 16, hw_])
                    nc.vector.tensor_tensor(out=t13, in0=t13, in1=mask_b,
                                            op=mybir.AluOpType.mult)
                nc.vector.tensor_reduce(out=acc_t[:], in_=t13,
                                        axis=mybir.AxisListType.X,
                                        op=mybir.AluOpType.add)
            nc.vector.tensor_tensor(out=acc_sb[:], in0=acc_sb[:],
                                    in1=acc2_sb[:], op=mybir.AluOpType.add)
            nc.scalar.dma_start(acc_o.ap(), acc_sb[:])
    nc.compile()
    return nc


# ---------------- pipeline ----------------

def run_pipeline(inputs, trace=False):
    import ml_dtypes
    exch_np = ml_dtypes.bfloat16 if EXCH_BF16 else np.float32

    x = np.asarray(inputs["x"]).reshape(-1).astype(np.float32)
    ei = np.asarray(inputs["edge_index"])
    src = ei[0].astype(np.int64)
    dst = ei[1].astype(np.int64)
    W1 = np.asarray(inputs["W1"]).astype(np.float64)[0]
    W2 = np.asarray(inputs["W2"]).astype(np.float64)
    b2 = np.asarray(inputs["b2"]).astype(np.float64)
    Wl = np.asarray(inputs["Wl"]).astype(np.float64)
    bl = np.asarray(inputs["bl"]).astype(np.float64)
    a_vec = np.maximum(W1, 0) @ W2
    b_vec = np.maximum(-W1, 0) @ W2

    xpad = np.zeros(NPAD, np.float32)
    xpad[:x.shape[0]] = x
    maskpad = np.zeros(NPAD, np.float32)
    maskpad[:x.shape[0]] = 1.0

    indeg_cnt = np.bincount(dst, minlength=NPAD).astype(np.int64)
    outdeg_cnt = np.bincount(src, minlength=NPAD).astype(np.int64)
    core_of, lane_of, q_of, KDs, KSs = _node_layout(indeg_cnt, outdeg_cnt)
    colD, CD = _col_bases(KDs)
    colS, CS = _col_bases(KSs)

    rank_d = _ranks(dst)
    rank_s = _ranks(src)
    dslot = (core_of[dst] * P + lane_of[dst]) * CD + colD[q_of[dst]] + rank_d
    sslot = (core_of[src] * P + lane_of[src]) * CS + colS[q_of[src]] + rank_s

    x_grids = _grid_scatter(core_of, lane_of, q_of, xpad)
    mask_grids = _grid_scatter(core_of, lane_of, q_of, maskpad)

    maskD = np.zeros(NC * P * CD, np.float32)
    maskD[dslot] = 1.0
    maskD = np.ascontiguousarray(
        maskD.reshape(NC, P, CD).astype(ml_dtypes.bfloat16))

    cvec = np.zeros(48, np.float32)
    cvec[0:16] = a_vec
    cvec[16:32] = b_vec
    cvec[32:48] = b2
    cvec = np.ascontiguousarray(np.tile(cvec, (P, 1)))

    phase_ns = {}

    def run(nc, in_maps, name):
        res = bass_utils.run_bass_kernel_spmd(
            nc, in_maps, core_ids=list(range(NC)), trace=trace)
        phase_ns[name] = res.exec_time_ns
        return res.results

    def exchange(m_stack):
        """Permute per-edge values: src-major layout -> dst-major layout."""
        m_flat = np.ascontiguousarray(m_stack).reshape(-1)
        v = np.zeros(NC * P * CD, exch_np)
        v[dslot] = m_flat[sslot]
        return v.reshape(NC, P, CD)

    ncA = build_pA(KDs, KSs)
    rA = run(ncA, [dict(maskD=maskD[kk], xg=x_grids[kk]) for kk in range(NC)],
             "pA")
    dinv_g = np.stack([rA[kk]["dinv"] for kk in range(NC)])
    u_g = np.stack([rA[kk]["u"] for kk in range(NC)])
    vD1 = exchange(np.stack([rA[kk]["m1"] for kk in range(NC)]))

    ncB = build_pB(KDs, KSs)
    rB = run(ncB, [dict(vD1=vD1[kk], u=u_g[kk], dinvg=dinv_g[kk])
                   for kk in range(NC)], "pB")
    y_g = np.stack([rB[kk]["yg"] for kk in range(NC)])
    vD2 = exchange(np.stack([rB[kk]["m2"] for kk in range(NC)]))

    ncC = build_pC(KDs, b2_zero=bool(np.all(b2 == 0.0)))
    rC = run(ncC, [dict(vD2=vD2[kk], dinvg=dinv_g[kk], yg=y_g[kk],
                        maskg=mask_grids[kk], cvec=cvec)
                   for kk in range(NC)], "pC")
    acc = np.stack([rC[kk]["acc"] for kk in range(NC)])

    pooled = acc.sum(axis=(0, 1)).astype(np.float64) / float(x.shape[0])
    logits = pooled @ Wl + bl
    m = logits.max()
    out = (logits - m) - np.log(np.exp(logits - m).sum())
    return out[None, :].astype(np.float32), phase_ns


def kernel(**inputs) -> np.ndarray:
    out, _ = run_pipeline(inputs, trace=False)
    return out
